# revision 10
# baseline (speedup 1.0000x reference)
"""AngularAttention (windowed cosine attention) Trainium2 kernel, 8-core data-parallel.

Per core = 2 images = 32 windows x 196 tokens. See stage comments inline.
"""
import sys

sys.path.insert(0, "/opt/trn_rl_repo")

import numpy as np

import concourse.bacc as bacc
import concourse.mybir as mybir
import concourse.tile as tile
from concourse.bass_utils import run_bass_kernel_spmd

F32 = mybir.dt.float32
F32R = mybir.dt.float32r
BF16 = mybir.dt.bfloat16
AF = mybir.ActivationFunctionType

NCORES = 8
NW = 32
L = 196
T = NW * L
E = 256
SCALE = 10.0

CHUNKS = [(i * 512, 512) for i in range(12)] + [(6144, 128)]
KC = [(0, 128), (128, 68)]


def _build():
    nc = bacc.Bacc(None)
    x = nc.declare_dram_parameter("x", [NW, L, E], F32, isOutput=False)
    w_qkv = nc.declare_dram_parameter("w_qkv", [2, 128, 768], F32R, isOutput=False)
    bqkT = nc.declare_dram_parameter("bqkT", [128, 4], F32, isOutput=False)
    w_proj = nc.declare_dram_parameter("w_proj", [2, 128, 256], F32R, isOutput=False)
    bv_bc = nc.declare_dram_parameter("bv_bc", [128, 256], F32, isOutput=False)
    bp_bc = nc.declare_dram_parameter("bp_bc", [128, 2, 256], F32, isOutput=False)
    ident = nc.declare_dram_parameter("ident", [128, 128], F32, isOutput=False)
    ind16 = nc.declare_dram_parameter("ind16", [128, 4, 16], F32R, isOutput=False)
    bcT = nc.declare_dram_parameter("bcT", [16, 4, 128], F32R, isOutput=False)
    onesb = nc.declare_dram_parameter("onesb", [128, 32], BF16, isOutput=False)
    out = nc.declare_dram_parameter("out", [NW, L, E], F32, isOutput=True)

    with tile.TileContext(nc) as tc:
        from contextlib import ExitStack

        with ExitStack() as root:
            const = root.enter_context(tc.tile_pool(name="const", bufs=1))
            # f32 arena: [0:4] bqkT, [4:132] ident, [132:388] bv, [388:900] bp
            sb_m = const.tile([128, 900], F32)
            nc.sync.dma_start(out=sb_m[:, 0:4], in_=bqkT[:, :])
            nc.sync.dma_start(out=sb_m[:, 4:132], in_=ident[:, :])
            nc.sync.dma_start(out=sb_m[:, 132:388], in_=bv_bc[:, :])
            nc.sync.dma_start(
                out=sb_m[:, 388:900], in_=bp_bc[:, :, :].rearrange("p a b -> p (a b)")
            )
            sb_bqkT = sb_m[:, 0:4]
            sb_id = sb_m[:, 4:132]
            sb_bv = sb_m[:, 132:388]
            sb_bp = sb_m[:, 388:900].rearrange("p (a b) -> p a b", a=2)
            # f32r arena: [0:1536] w_qkv (2,768), [1536:1600] ind16 (4,16), [1600:2112] w_proj (2,256)
            sb_r = const.tile([128, 2112], F32R)
            nc.sync.dma_start(
                out=sb_r[:, 0:1536].rearrange("p (c f) -> p c f", c=2),
                in_=w_qkv[:, :, :].rearrange("c p f -> p c f"),
            )
            nc.sync.dma_start(
                out=sb_r[:, 1536:1600], in_=ind16[:, :, :].rearrange("p a b -> p (a b)")
            )
            nc.sync.dma_start(
                out=sb_r[:, 1600:2112].rearrange("p (c f) -> p c f", c=2),
                in_=w_proj[:, :, :].rearrange("c p f -> p c f"),
            )
            sb_wqkv = sb_r[:, 0:1536].rearrange("p (c f) -> p c f", c=2)
            sb_i16 = sb_r[:, 1536:1600].rearrange("p (a b) -> p a b", a=4)
            sb_wproj = sb_r[:, 1600:2112].rearrange("p (c f) -> p c f", c=2)

            sb_bcT = const.tile([16, 4, 128], F32R)
            nc.sync.dma_start(out=sb_bcT[:], in_=bcT[:, :, :])
            sb_ones = const.tile([128, 32], BF16)
            nc.sync.dma_start(out=sb_ones[:], in_=onesb[:, :])

            big = root.enter_context(tc.tile_pool(name="big", bufs=1))
            qkN = big.tile([128, 4, T], BF16)        # q heads 0-3 | q 4-7 | k 0-3 | k 4-7
            V0 = big.tile([128, NW, 256], BF16)      # k-chunk0 of each window, [tok, (head d)]
            V1 = big.tile([68, NW, 256], BF16)
            Vt = {0: V0, 1: V1}

            # ---------------- Stage A ----------------
            with ExitStack() as sa:
                xin = sa.enter_context(tc.tile_pool(name="xin", bufs=3))
                qkps = sa.enter_context(tc.tile_pool(name="qkps", bufs=2, space="PSUM"))
                ssps = sa.enter_context(tc.tile_pool(name="ssps", bufs=1, space="PSUM"))
                bcps = sa.enter_context(tc.tile_pool(name="bcps", bufs=2, space="PSUM"))
                vps = sa.enter_context(tc.tile_pool(name="vps", bufs=1, space="PSUM"))
                xtp = sa.enter_context(tc.tile_pool(name="xtp", bufs=1))
                rawp = sa.enter_context(tc.tile_pool(name="rawp", bufs=2))
                sqp = sa.enter_context(tc.tile_pool(name="sqp", bufs=1))
                nmp = sa.enter_context(tc.tile_pool(name="nmp", bufs=1))

                xT = xtp.tile([128, 2, T], F32R)

                # A1: load x + PE-transpose, 4 halves (98 tok) per psum batch
                for b0 in range(0, NW * 2, 4):
                    ptr = [
                        qkps.tile([128, 4, 128], F32, tag="qkps", name=f"tr{e}")
                        for e in range(2)
                    ]
                    for s in range(4):
                        hf = b0 + s
                        w, th = hf // 2, hf % 2
                        xt_in = xin.tile([98, 256], F32)
                        nc.sync.dma_start(out=xt_in[:], in_=x[w, 98 * th : 98 * th + 98, :])
                        for e in range(2):
                            nc.tensor.transpose(
                                ptr[e][:, s, 0:98],
                                xt_in[:, 128 * e : 128 * e + 128],
                                sb_id[0:98, 0:98],
                            )
                    for e in range(2):
                        nc.vector.tensor_copy(
                            xT[:, e, 98 * b0 : 98 * (b0 + 4)], ptr[e][:, :, 0:98]
                        )

                # A2: qkT (f32r) + cosine normalization, per token chunk
                for c0, cs in CHUNKS:
                    qk_ps = qkps.tile([128, 2, 512], F32, tag="qkps", name="qkps")
                    ss_ps = ssps.tile([16, 512], F32, tag="ssps", name="ssps")
                    raw = rawp.tile([128, 4, 512], BF16, tag="raw", name="raw")
                    sq = sqp.tile([128, 4, 512], F32R, tag="sq", name="sq")
                    for ft in range(4):
                        slot = ft % 2
                        for e in range(2):
                            nc.tensor.matmul(
                                qk_ps[:, slot, 0:cs],
                                sb_wqkv[:, e, 128 * ft : 128 * ft + 128],
                                xT[:, e, c0 : c0 + cs],
                                start=(e == 0),
                                stop=(e == 1),
                            )
                        nc.scalar.activation(
                            raw[:, ft, 0:cs], qk_ps[:, slot, 0:cs], AF.Identity,
                            bias=sb_bqkT[:, ft : ft + 1], scale=1.0,
                        )
                        nc.gpsimd.tensor_tensor(
                            out=sq[:, ft, 0:cs], in0=raw[:, ft, 0:cs], in1=raw[:, ft, 0:cs],
                            op=mybir.AluOpType.mult,
                        )
                        nc.tensor.matmul(
                            ss_ps[:, 0:cs], sb_i16[:, ft, :], sq[:, ft, 0:cs],
                            start=(ft == 0), stop=(ft == 3),
                        )
                    snorm = nmp.tile([16, 512], F32, tag="snorm", name="snorm")
                    nc.scalar.activation(snorm[:, 0:cs], ss_ps[:, 0:cs], AF.Ln, scale=1.0)
                    invn = nmp.tile([16, 512], F32R, tag="invn", name="invn")
                    nc.scalar.activation(invn[:, 0:cs], snorm[:, 0:cs], AF.Exp, scale=-0.5)
                    for ft in range(4):
                        bc_ps = bcps.tile([128, 512], F32, tag="bcps", name="bcps")
                        nc.tensor.matmul(
                            bc_ps[:, 0:cs], sb_bcT[:, ft, :], invn[:, 0:cs],
                            start=True, stop=True,
                        )
                        nc.vector.tensor_tensor(
                            out=qkN[:, ft, c0 : c0 + cs], in0=raw[:, ft, 0:cs],
                            in1=bc_ps[:, 0:cs], op=mybir.AluOpType.mult,
                        )

                # A3: V natural (f32r matmuls from xT as stationary)
                for w in range(NW):
                    for ci, (k0, ks) in enumerate(KC):
                        v_ps = vps.tile([128, 256], F32, tag="vps", name="vps")
                        for e in range(2):
                            nc.tensor.matmul(
                                v_ps[0:ks, :],
                                xT[:, e, w * L + k0 : w * L + k0 + ks],
                                sb_wqkv[:, e, 512:768],
                                start=(e == 0), stop=(e == 1),
                            )
                        nc.vector.scalar_tensor_tensor(
                            out=Vt[ci][:, w, :], in0=v_ps[0:ks, :], scalar=1.0,
                            in1=sb_bv[0:ks, :],
                            op0=mybir.AluOpType.mult, op1=mybir.AluOpType.add,
                        )

            # ---------------- Stage B ----------------
            with ExitStack() as sb_:
                sps = sb_.enter_context(tc.tile_pool(name="sps", bufs=1, space="PSUM"))
                aps = sb_.enter_context(tc.tile_pool(name="aps", bufs=1, space="PSUM"))
                bps = sb_.enter_context(tc.tile_pool(name="bps", bufs=1, space="PSUM"))
                yps = sb_.enter_context(tc.tile_pool(name="yps", bufs=2, space="PSUM"))
                expp = sb_.enter_context(tc.tile_pool(name="expp", bufs=3))
                etp = sb_.enter_context(tc.tile_pool(name="etp", bufs=2))
                invp = sb_.enter_context(tc.tile_pool(name="invp", bufs=2))
                ysb = sb_.enter_context(tc.tile_pool(name="ysb", bufs=2))

                Vv = {ci: Vt[ci][:].rearrange("p w (h d) -> p w h d", h=8) for ci in (0, 1)}

                for wp in range(NW // 2):
                    expT = {}
                    for wi in range(2):
                        w = 2 * wp + wi
                        for ci, (k0, ks) in enumerate(KC):
                            et = expp.tile(
                                [ks, 8, L], BF16, tag=f"exp{wi}_{ci}", name=f"exp{wi}_{ci}"
                            )
                            expT[(wi, ci)] = et
                            for g in range(2):
                                s_ps = sps.tile([128, 4, 512], F32, tag="sps", name="sps")
                                for hp in range(4):
                                    nc.tensor.matmul(
                                        s_ps[0:ks, hp, 0:L],
                                        qkN[32 * hp : 32 * hp + 32, 2 + g,
                                            w * L + k0 : w * L + k0 + ks],
                                        qkN[32 * hp : 32 * hp + 32, g, w * L : w * L + L],
                                        start=True, stop=True,
                                        tile_position=(32 * hp, 0),
                                    )
                                nc.scalar.activation(
                                    et[:, 4 * g : 4 * g + 4, :], s_ps[0:ks, :, 0:L],
                                    AF.Exp, scale=SCALE,
                                )
                    eTs = []
                    for hb in range(2):
                        pA = aps.tile([128, 2, L], F32, tag="pA", name="pA")
                        pB = bps.tile([128, 2, L], F32, tag="pB", name="pB")
                        for wi in range(2):
                            w = 2 * wp + wi
                            for hp in range(4):
                                h = 4 * hb + hp
                                for ci, (k0, ks) in enumerate(KC):
                                    nc.tensor.matmul(
                                        pA[32 * hp : 32 * hp + 32, wi, :],
                                        Vv[ci][:, w, h, :],
                                        expT[(wi, ci)][:, h, :],
                                        start=(ci == 0), stop=(ci == 1),
                                        tile_position=(0, 32 * hp),
                                    )
                                    nc.tensor.matmul(
                                        pB[32 * hp : 32 * hp + 32, wi, :],
                                        sb_ones[0:ks, :],
                                        expT[(wi, ci)][:, h, :],
                                        start=(ci == 0), stop=(ci == 1),
                                        tile_position=(0, 32 * hp),
                                    )
                        lnd = invp.tile([128, 2, L], F32, tag="lnd", name="lnd")
                        nc.scalar.activation(lnd[:], pB[:], AF.Ln, scale=1.0)
                        inv = invp.tile([128, 2, L], F32, tag="inv", name="inv")
                        nc.scalar.activation(inv[:], lnd[:], AF.Exp, scale=-1.0)
                        eT = etp.tile([128, 2, L], F32R, tag=f"eT{hb}", name=f"eT{hb}")
                        eTs.append(eT)
                        with nc.allow_low_precision(reason="attn out f32r for proj"):
                            nc.vector.tensor_tensor(
                                out=eT[:], in0=pA[:], in1=inv[:], op=mybir.AluOpType.mult
                            )
                    for wi in range(2):
                        w = 2 * wp + wi
                        y_ps = yps.tile([98, 2, 256], F32, tag="yps", name="yps")
                        for th in range(2):
                            for hb in range(2):
                                nc.tensor.matmul(
                                    y_ps[:, th, :],
                                    eTs[hb][:, wi, 98 * th : 98 * th + 98],
                                    sb_wproj[:, hb, :],
                                    start=(hb == 0), stop=(hb == 1),
                                )
                        y_sb = ysb.tile([98, 2, 256], F32, tag="ysb", name="ysb")
                        nc.vector.scalar_tensor_tensor(
                            out=y_sb[:], in0=y_ps[:], scalar=1.0, in1=sb_bp[0:98, :, :],
                            op0=mybir.AluOpType.mult, op1=mybir.AluOpType.add,
                        )
                        nc.sync.dma_start(
                            out=out[w, :, :].rearrange("(th p) e -> p th e", th=2),
                            in_=y_sb[:],
                        )

    nc.finalize()
    return nc


_NC = None


def _get_nc():
    global _NC
    if _NC is None:
        _NC = _build()
    return _NC


def _consts():
    import ml_dtypes

    p = np.arange(128)
    ind16 = np.zeros((128, 4, 16), np.float32)
    bcT = np.zeros((16, 4, 128), np.float32)
    for ft in range(4):
        ind16[p, ft, 4 * ft + p // 32] = 1.0
        bcT[4 * ft + p // 32, ft, p] = 1.0
    return {
        "ident": np.eye(128, dtype=np.float32),
        "ind16": ind16,
        "bcT": bcT,
        "onesb": np.ones((128, 32), ml_dtypes.bfloat16),
    }


def kernel(x, w_qkv, b_qkv, w_proj, b_proj, _trace=False):
    x = np.ascontiguousarray(np.asarray(x, np.float32))
    w_qkv = np.asarray(w_qkv, np.float32)
    b_qkv = np.asarray(b_qkv, np.float32)
    w_proj = np.asarray(w_proj, np.float32)
    b_proj = np.asarray(b_proj, np.float32)

    B, H, W, _ = x.shape
    xw = (
        x.reshape(NCORES, 2, 4, 14, 4, 14, E)
        .transpose(0, 1, 2, 4, 3, 5, 6)
        .reshape(NCORES, NW, L, E)
    )
    base = {
        "w_qkv": np.ascontiguousarray(w_qkv.reshape(2, 128, 768)),
        "bqkT": np.ascontiguousarray(b_qkv[:512].reshape(4, 128).T),
        "w_proj": np.ascontiguousarray(w_proj.reshape(2, 128, 256)),
        "bv_bc": np.broadcast_to(b_qkv[512:768], (128, 256)).copy(),
        "bp_bc": np.broadcast_to(b_proj, (128, 2, 256)).copy(),
        **_consts(),
    }
    in_maps = [dict(base, x=np.ascontiguousarray(xw[c])) for c in range(NCORES)]

    nc = _get_nc()
    br = run_bass_kernel_spmd(nc, in_maps, list(range(NCORES)), trace=_trace)
    outs = np.stack([br.results[c]["out"] for c in range(NCORES)])
    y = (
        outs.reshape(NCORES, 2, 4, 4, 14, 14, E)
        .transpose(0, 1, 2, 4, 3, 5, 6)
        .reshape(B, H, W, E)
    )
    if _trace:
        return y, br.exec_time_ns
    return y


# revision 12
# speedup vs baseline: 1.0374x; 1.0374x over previous
"""AngularAttention (windowed cosine attention) Trainium2 kernel, 8-core data-parallel.

Per core = 2 images = 32 windows x 196 tokens. See stage comments inline.
"""
import sys

sys.path.insert(0, "/opt/trn_rl_repo")

import numpy as np

import concourse.bacc as bacc
import concourse.mybir as mybir
import concourse.tile as tile
from concourse.bass_utils import run_bass_kernel_spmd

F32 = mybir.dt.float32
F32R = mybir.dt.float32r
BF16 = mybir.dt.bfloat16
AF = mybir.ActivationFunctionType

NCORES = 8
NW = 32
L = 196
T = NW * L
E = 256
SCALE = 10.0

CHUNKS = [(i * 512, 512) for i in range(12)] + [(6144, 128)]
KC = [(0, 128), (128, 68)]


def _build():
    nc = bacc.Bacc(None)
    x = nc.declare_dram_parameter("x", [NW, L, E], F32, isOutput=False)
    w_qkv = nc.declare_dram_parameter("w_qkv", [2, 128, 768], F32R, isOutput=False)
    bqkT = nc.declare_dram_parameter("bqkT", [128, 4], F32, isOutput=False)
    w_proj = nc.declare_dram_parameter("w_proj", [2, 128, 256], F32R, isOutput=False)
    bv_bc = nc.declare_dram_parameter("bv_bc", [128, 256], F32, isOutput=False)
    bp_bc = nc.declare_dram_parameter("bp_bc", [128, 2, 256], F32, isOutput=False)
    ident = nc.declare_dram_parameter("ident", [128, 128], F32, isOutput=False)
    ind16 = nc.declare_dram_parameter("ind16", [128, 4, 16], F32R, isOutput=False)
    bcT = nc.declare_dram_parameter("bcT", [16, 4, 128], F32R, isOutput=False)
    onesb = nc.declare_dram_parameter("onesb", [128, 32], BF16, isOutput=False)
    out = nc.declare_dram_parameter("out", [NW, L, E], F32, isOutput=True)

    with tile.TileContext(nc) as tc:
        from contextlib import ExitStack

        with ExitStack() as root:
            const = root.enter_context(tc.tile_pool(name="const", bufs=1))
            # f32 arena: [0:4] bqkT, [4:132] ident, [132:388] bv, [388:900] bp
            sb_m = const.tile([128, 900], F32)
            nc.sync.dma_start(out=sb_m[:, 0:4], in_=bqkT[:, :])
            nc.sync.dma_start(out=sb_m[:, 4:132], in_=ident[:, :])
            nc.sync.dma_start(out=sb_m[:, 132:388], in_=bv_bc[:, :])
            nc.sync.dma_start(
                out=sb_m[:, 388:900], in_=bp_bc[:, :, :].rearrange("p a b -> p (a b)")
            )
            sb_bqkT = sb_m[:, 0:4]
            sb_id = sb_m[:, 4:132]
            sb_bv = sb_m[:, 132:388]
            sb_bp = sb_m[:, 388:900].rearrange("p (a b) -> p a b", a=2)
            # f32r arena: [0:1536] w_qkv (2,768), [1536:1600] ind16 (4,16), [1600:2112] w_proj (2,256)
            sb_r = const.tile([128, 2112], F32R)
            nc.sync.dma_start(
                out=sb_r[:, 0:1536].rearrange("p (c f) -> p c f", c=2),
                in_=w_qkv[:, :, :].rearrange("c p f -> p c f"),
            )
            nc.sync.dma_start(
                out=sb_r[:, 1536:1600], in_=ind16[:, :, :].rearrange("p a b -> p (a b)")
            )
            nc.sync.dma_start(
                out=sb_r[:, 1600:2112].rearrange("p (c f) -> p c f", c=2),
                in_=w_proj[:, :, :].rearrange("c p f -> p c f"),
            )
            sb_wqkv = sb_r[:, 0:1536].rearrange("p (c f) -> p c f", c=2)
            sb_i16 = sb_r[:, 1536:1600].rearrange("p (a b) -> p a b", a=4)
            sb_wproj = sb_r[:, 1600:2112].rearrange("p (c f) -> p c f", c=2)

            sb_bcT = const.tile([16, 4, 128], F32R)
            nc.sync.dma_start(out=sb_bcT[:], in_=bcT[:, :, :])
            sb_ones = const.tile([128, 32], BF16)
            nc.sync.dma_start(out=sb_ones[:], in_=onesb[:, :])

            big = root.enter_context(tc.tile_pool(name="big", bufs=1))
            qkN = big.tile([128, 4, T], BF16)        # q heads 0-3 | q 4-7 | k 0-3 | k 4-7
            V0 = big.tile([128, NW, 256], BF16)      # k-chunk0 of each window, [tok, (head d)]
            V1 = big.tile([68, NW, 256], BF16)
            Vt = {0: V0, 1: V1}

            # ---------------- Stage A ----------------
            with ExitStack() as sa:
                xin = sa.enter_context(tc.tile_pool(name="xin", bufs=3))
                qkps = sa.enter_context(tc.tile_pool(name="qkps", bufs=2, space="PSUM"))
                ssps = sa.enter_context(tc.tile_pool(name="ssps", bufs=1, space="PSUM"))
                bcps = sa.enter_context(tc.tile_pool(name="bcps", bufs=2, space="PSUM"))
                vps = sa.enter_context(tc.tile_pool(name="vps", bufs=1, space="PSUM"))
                xtp = sa.enter_context(tc.tile_pool(name="xtp", bufs=1))
                rawp = sa.enter_context(tc.tile_pool(name="rawp", bufs=2))
                sqp = sa.enter_context(tc.tile_pool(name="sqp", bufs=1))
                nmp = sa.enter_context(tc.tile_pool(name="nmp", bufs=1))

                xT = xtp.tile([128, 2, T], F32R)

                # A1: load x + PE-transpose, 4 halves (98 tok) per psum batch
                for b0 in range(0, NW * 2, 4):
                    ptr = [
                        qkps.tile([128, 4, 128], F32, tag="qkps", name=f"tr{e}")
                        for e in range(2)
                    ]
                    for s in range(4):
                        hf = b0 + s
                        w, th = hf // 2, hf % 2
                        xt_in = xin.tile([98, 256], F32)
                        nc.sync.dma_start(out=xt_in[:], in_=x[w, 98 * th : 98 * th + 98, :])
                        for e in range(2):
                            nc.tensor.transpose(
                                ptr[e][:, s, 0:98],
                                xt_in[:, 128 * e : 128 * e + 128],
                                sb_id[0:98, 0:98],
                            )
                    for e in range(2):
                        nc.vector.tensor_copy(
                            xT[:, e, 98 * b0 : 98 * (b0 + 4)], ptr[e][:, :, 0:98]
                        )

                # A2: qkT (f32r) + cosine normalization, per token chunk
                for c0, cs in CHUNKS:
                    qk_ps = qkps.tile([128, 2, 512], F32, tag="qkps", name="qkps")
                    ss_ps = ssps.tile([16, 512], F32, tag="ssps", name="ssps")
                    raw = rawp.tile([128, 4, 512], BF16, tag="raw", name="raw")
                    sq = sqp.tile([128, 4, 512], F32R, tag="sq", name="sq")
                    for ft in range(4):
                        slot = ft % 2
                        for e in range(2):
                            nc.tensor.matmul(
                                qk_ps[:, slot, 0:cs],
                                sb_wqkv[:, e, 128 * ft : 128 * ft + 128],
                                xT[:, e, c0 : c0 + cs],
                                start=(e == 0),
                                stop=(e == 1),
                            )
                        nc.scalar.activation(
                            raw[:, ft, 0:cs], qk_ps[:, slot, 0:cs], AF.Identity,
                            bias=sb_bqkT[:, ft : ft + 1], scale=1.0,
                        )
                        nc.gpsimd.tensor_tensor(
                            out=sq[:, ft, 0:cs], in0=raw[:, ft, 0:cs], in1=raw[:, ft, 0:cs],
                            op=mybir.AluOpType.mult,
                        )
                        nc.tensor.matmul(
                            ss_ps[:, 0:cs], sb_i16[:, ft, :], sq[:, ft, 0:cs],
                            start=(ft == 0), stop=(ft == 3),
                        )
                    rss = nmp.tile([16, 512], F32, tag="rss", name="rss")
                    nc.vector.reciprocal_approx_fast(out=rss[:, 0:cs], in_=ss_ps[:, 0:cs])
                    invn = nmp.tile([16, 512], F32R, tag="invn", name="invn")
                    nc.scalar.activation(invn[:, 0:cs], rss[:, 0:cs], AF.Sqrt, scale=1.0)
                    for ft in range(4):
                        bc_ps = bcps.tile([128, 512], F32, tag="bcps", name="bcps")
                        nc.tensor.matmul(
                            bc_ps[:, 0:cs], sb_bcT[:, ft, :], invn[:, 0:cs],
                            start=True, stop=True,
                        )
                        nc.vector.tensor_tensor(
                            out=qkN[:, ft, c0 : c0 + cs], in0=raw[:, ft, 0:cs],
                            in1=bc_ps[:, 0:cs], op=mybir.AluOpType.mult,
                        )

                # A3: V natural (f32r matmuls from xT as stationary)
                for w in range(NW):
                    for ci, (k0, ks) in enumerate(KC):
                        v_ps = vps.tile([128, 256], F32, tag="vps", name="vps")
                        for e in range(2):
                            nc.tensor.matmul(
                                v_ps[0:ks, :],
                                xT[:, e, w * L + k0 : w * L + k0 + ks],
                                sb_wqkv[:, e, 512:768],
                                start=(e == 0), stop=(e == 1),
                            )
                        nc.vector.scalar_tensor_tensor(
                            out=Vt[ci][:, w, :], in0=v_ps[0:ks, :], scalar=1.0,
                            in1=sb_bv[0:ks, :],
                            op0=mybir.AluOpType.mult, op1=mybir.AluOpType.add,
                        )

            # ---------------- Stage B ----------------
            with ExitStack() as sb_:
                sps = sb_.enter_context(tc.tile_pool(name="sps", bufs=1, space="PSUM"))
                aps = sb_.enter_context(tc.tile_pool(name="aps", bufs=1, space="PSUM"))
                bps = sb_.enter_context(tc.tile_pool(name="bps", bufs=1, space="PSUM"))
                yps = sb_.enter_context(tc.tile_pool(name="yps", bufs=2, space="PSUM"))
                expp = sb_.enter_context(tc.tile_pool(name="expp", bufs=3))
                etp = sb_.enter_context(tc.tile_pool(name="etp", bufs=2))
                invp = sb_.enter_context(tc.tile_pool(name="invp", bufs=2))
                ysb = sb_.enter_context(tc.tile_pool(name="ysb", bufs=2))

                Vv = {ci: Vt[ci][:].rearrange("p w (h d) -> p w h d", h=8) for ci in (0, 1)}

                def emit_s_exp(wp):
                    expT = {}
                    for wi in range(2):
                        w = 2 * wp + wi
                        for ci, (k0, ks) in enumerate(KC):
                            et = expp.tile(
                                [ks, 8, L], BF16, tag=f"exp{wi}_{ci}", name=f"exp{wi}_{ci}"
                            )
                            expT[(wi, ci)] = et
                            for g in range(2):
                                s_ps = sps.tile([128, 4, 512], F32, tag="sps", name="sps")
                                for hp in range(4):
                                    nc.tensor.matmul(
                                        s_ps[0:ks, hp, 0:L],
                                        qkN[32 * hp : 32 * hp + 32, 2 + g,
                                            w * L + k0 : w * L + k0 + ks],
                                        qkN[32 * hp : 32 * hp + 32, g, w * L : w * L + L],
                                        start=True, stop=True,
                                        tile_position=(32 * hp, 0),
                                    )
                                nc.scalar.activation(
                                    et[:, 4 * g : 4 * g + 4, :], s_ps[0:ks, :, 0:L],
                                    AF.Exp, scale=SCALE,
                                )
                    return expT

                def emit_avt_proj(wp, expT):
                    eTs = []
                    for hb in range(2):
                        pA = aps.tile([128, 2, L], F32, tag="pA", name="pA")
                        pB = bps.tile([128, 2, L], F32, tag="pB", name="pB")
                        for wi in range(2):
                            w = 2 * wp + wi
                            for hp in range(4):
                                h = 4 * hb + hp
                                for ci, (k0, ks) in enumerate(KC):
                                    nc.tensor.matmul(
                                        pA[32 * hp : 32 * hp + 32, wi, :],
                                        Vv[ci][:, w, h, :],
                                        expT[(wi, ci)][:, h, :],
                                        start=(ci == 0), stop=(ci == 1),
                                        tile_position=(0, 32 * hp),
                                    )
                                    nc.tensor.matmul(
                                        pB[32 * hp : 32 * hp + 32, wi, :],
                                        sb_ones[0:ks, :],
                                        expT[(wi, ci)][:, h, :],
                                        start=(ci == 0), stop=(ci == 1),
                                        tile_position=(0, 32 * hp),
                                    )
                        inv = invp.tile([128, 2, L], F32, tag="inv", name="inv")
                        nc.vector.reciprocal_approx_fast(out=inv[:], in_=pB[:])
                        eT = etp.tile([128, 2, L], F32R, tag=f"eT{hb}", name=f"eT{hb}")
                        eTs.append(eT)
                        with nc.allow_low_precision(reason="attn out f32r for proj"):
                            nc.vector.tensor_tensor(
                                out=eT[:], in0=pA[:], in1=inv[:], op=mybir.AluOpType.mult
                            )
                    for wi in range(2):
                        w = 2 * wp + wi
                        y_ps = yps.tile([98, 2, 256], F32, tag="yps", name="yps")
                        for th in range(2):
                            for hb in range(2):
                                nc.tensor.matmul(
                                    y_ps[:, th, :],
                                    eTs[hb][:, wi, 98 * th : 98 * th + 98],
                                    sb_wproj[:, hb, :],
                                    start=(hb == 0), stop=(hb == 1),
                                )
                        y_sb = ysb.tile([98, 2, 256], F32, tag="ysb", name="ysb")
                        nc.vector.scalar_tensor_tensor(
                            out=y_sb[:], in0=y_ps[:], scalar=1.0, in1=sb_bp[0:98, :, :],
                            op0=mybir.AluOpType.mult, op1=mybir.AluOpType.add,
                        )
                        nc.sync.dma_start(
                            out=out[w, :, :].rearrange("(th p) e -> p th e", th=2),
                            in_=y_sb[:],
                        )

                prev = None
                for wp in range(NW // 2):
                    cur = emit_s_exp(wp)
                    if prev is not None:
                        emit_avt_proj(wp - 1, prev)
                    prev = cur
                emit_avt_proj(NW // 2 - 1, prev)

    nc.finalize()
    return nc


_NC = None


def _get_nc():
    global _NC
    if _NC is None:
        _NC = _build()
    return _NC


def _consts():
    import ml_dtypes

    p = np.arange(128)
    ind16 = np.zeros((128, 4, 16), np.float32)
    bcT = np.zeros((16, 4, 128), np.float32)
    for ft in range(4):
        ind16[p, ft, 4 * ft + p // 32] = 1.0
        bcT[4 * ft + p // 32, ft, p] = 1.0
    return {
        "ident": np.eye(128, dtype=np.float32),
        "ind16": ind16,
        "bcT": bcT,
        "onesb": np.ones((128, 32), ml_dtypes.bfloat16),
    }


def kernel(x, w_qkv, b_qkv, w_proj, b_proj, _trace=False):
    x = np.ascontiguousarray(np.asarray(x, np.float32))
    w_qkv = np.asarray(w_qkv, np.float32)
    b_qkv = np.asarray(b_qkv, np.float32)
    w_proj = np.asarray(w_proj, np.float32)
    b_proj = np.asarray(b_proj, np.float32)

    B, H, W, _ = x.shape
    xw = (
        x.reshape(NCORES, 2, 4, 14, 4, 14, E)
        .transpose(0, 1, 2, 4, 3, 5, 6)
        .reshape(NCORES, NW, L, E)
    )
    base = {
        "w_qkv": np.ascontiguousarray(w_qkv.reshape(2, 128, 768)),
        "bqkT": np.ascontiguousarray(b_qkv[:512].reshape(4, 128).T),
        "w_proj": np.ascontiguousarray(w_proj.reshape(2, 128, 256)),
        "bv_bc": np.broadcast_to(b_qkv[512:768], (128, 256)).copy(),
        "bp_bc": np.broadcast_to(b_proj, (128, 2, 256)).copy(),
        **_consts(),
    }
    in_maps = [dict(base, x=np.ascontiguousarray(xw[c])) for c in range(NCORES)]

    nc = _get_nc()
    br = run_bass_kernel_spmd(nc, in_maps, list(range(NCORES)), trace=_trace)
    outs = np.stack([br.results[c]["out"] for c in range(NCORES)])
    y = (
        outs.reshape(NCORES, 2, 4, 4, 14, 14, E)
        .transpose(0, 1, 2, 4, 3, 5, 6)
        .reshape(B, H, W, E)
    )
    if _trace:
        return y, br.exec_time_ns
    return y


# revision 13
# speedup vs baseline: 1.1072x; 1.0673x over previous
"""AngularAttention (windowed cosine attention) Trainium2 kernel, 8-core data-parallel.

Per core = 2 images = 32 windows x 196 tokens. See stage comments inline.
"""
import sys

sys.path.insert(0, "/opt/trn_rl_repo")

import numpy as np

import concourse.bacc as bacc
import concourse.mybir as mybir
import concourse.tile as tile
from concourse.bass_utils import run_bass_kernel_spmd

F32 = mybir.dt.float32
F32R = mybir.dt.float32r
BF16 = mybir.dt.bfloat16
AF = mybir.ActivationFunctionType

NCORES = 8
NW = 32
L = 196
T = NW * L
E = 256
SCALE = 10.0

CHUNKS = [(i * 512, 512) for i in range(12)] + [(6144, 128)]
KC = [(0, 128), (128, 68)]


def _build():
    nc = bacc.Bacc(None)
    x = nc.declare_dram_parameter("x", [NW, L, E], F32, isOutput=False)
    w_qkv = nc.declare_dram_parameter("w_qkv", [2, 128, 768], F32R, isOutput=False)
    bqkT = nc.declare_dram_parameter("bqkT", [128, 4], F32, isOutput=False)
    w_proj = nc.declare_dram_parameter("w_proj", [2, 128, 256], F32R, isOutput=False)
    bv_bc = nc.declare_dram_parameter("bv_bc", [128, 256], F32, isOutput=False)
    bp_bc = nc.declare_dram_parameter("bp_bc", [128, 2, 256], F32, isOutput=False)
    ident = nc.declare_dram_parameter("ident", [128, 128], F32, isOutput=False)
    ind16 = nc.declare_dram_parameter("ind16", [128, 4, 16], F32R, isOutput=False)
    bcT = nc.declare_dram_parameter("bcT", [16, 4, 128], F32R, isOutput=False)
    onesb = nc.declare_dram_parameter("onesb", [128, 32], BF16, isOutput=False)
    out = nc.declare_dram_parameter("out", [NW, L, E], F32, isOutput=True)

    with tile.TileContext(nc) as tc:
        from contextlib import ExitStack

        with ExitStack() as root:
            const = root.enter_context(tc.tile_pool(name="const", bufs=1))
            # f32 arena: [0:4] bqkT, [4:132] ident, [132:388] bv, [388:900] bp
            sb_m = const.tile([128, 900], F32)
            nc.sync.dma_start(out=sb_m[:, 0:4], in_=bqkT[:, :])
            nc.sync.dma_start(out=sb_m[:, 4:132], in_=ident[:, :])
            nc.sync.dma_start(out=sb_m[:, 132:388], in_=bv_bc[:, :])
            nc.sync.dma_start(
                out=sb_m[:, 388:900], in_=bp_bc[:, :, :].rearrange("p a b -> p (a b)")
            )
            sb_bqkT = sb_m[:, 0:4]
            sb_id = sb_m[:, 4:132]
            sb_bv = sb_m[:, 132:388]
            sb_bp = sb_m[:, 388:900].rearrange("p (a b) -> p a b", a=2)
            # f32r arena: [0:1536] w_qkv (2,768), [1536:1600] ind16 (4,16), [1600:2112] w_proj (2,256)
            sb_r = const.tile([128, 2112], F32R)
            nc.sync.dma_start(
                out=sb_r[:, 0:1536].rearrange("p (c f) -> p c f", c=2),
                in_=w_qkv[:, :, :].rearrange("c p f -> p c f"),
            )
            nc.sync.dma_start(
                out=sb_r[:, 1536:1600], in_=ind16[:, :, :].rearrange("p a b -> p (a b)")
            )
            nc.sync.dma_start(
                out=sb_r[:, 1600:2112].rearrange("p (c f) -> p c f", c=2),
                in_=w_proj[:, :, :].rearrange("c p f -> p c f"),
            )
            sb_wqkv = sb_r[:, 0:1536].rearrange("p (c f) -> p c f", c=2)
            sb_i16 = sb_r[:, 1536:1600].rearrange("p (a b) -> p a b", a=4)
            sb_wproj = sb_r[:, 1600:2112].rearrange("p (c f) -> p c f", c=2)

            sb_bcT = const.tile([16, 4, 128], F32R)
            nc.sync.dma_start(out=sb_bcT[:], in_=bcT[:, :, :])
            sb_ones = const.tile([128, 32], BF16)
            nc.sync.dma_start(out=sb_ones[:], in_=onesb[:, :])

            big = root.enter_context(tc.tile_pool(name="big", bufs=1))
            qkN = big.tile([128, 4, T], BF16)        # q heads 0-3 | q 4-7 | k 0-3 | k 4-7
            V0 = big.tile([128, NW, 256], BF16)      # k-chunk0 of each window, [tok, (head d)]
            V1 = big.tile([68, NW, 256], BF16)
            Vt = {0: V0, 1: V1}

            # ---------------- Stage A ----------------
            with ExitStack() as sa:
                xin = sa.enter_context(tc.tile_pool(name="xin", bufs=3))
                qkps = sa.enter_context(tc.tile_pool(name="qkps", bufs=2, space="PSUM"))
                ssps = sa.enter_context(tc.tile_pool(name="ssps", bufs=1, space="PSUM"))
                bcps = sa.enter_context(tc.tile_pool(name="bcps", bufs=2, space="PSUM"))
                vps = sa.enter_context(tc.tile_pool(name="vps", bufs=1, space="PSUM"))
                xtp = sa.enter_context(tc.tile_pool(name="xtp", bufs=1))
                rawp = sa.enter_context(tc.tile_pool(name="rawp", bufs=3))
                sqp = sa.enter_context(tc.tile_pool(name="sqp", bufs=2))
                nmp = sa.enter_context(tc.tile_pool(name="nmp", bufs=2))

                xT = xtp.tile([128, 2, T], F32R)

                # A1: load x + PE-transpose, 4 halves (98 tok) per psum batch
                for b0 in range(0, NW * 2, 4):
                    ptr = [
                        qkps.tile([128, 4, 128], F32, tag="qkps", name=f"tr{e}")
                        for e in range(2)
                    ]
                    for s in range(4):
                        hf = b0 + s
                        w, th = hf // 2, hf % 2
                        xt_in = xin.tile([98, 256], F32)
                        nc.sync.dma_start(out=xt_in[:], in_=x[w, 98 * th : 98 * th + 98, :])
                        for e in range(2):
                            nc.tensor.transpose(
                                ptr[e][:, s, 0:98],
                                xt_in[:, 128 * e : 128 * e + 128],
                                sb_id[0:98, 0:98],
                            )
                    for e in range(2):
                        nc.vector.tensor_copy(
                            xT[:, e, 98 * b0 : 98 * (b0 + 4)], ptr[e][:, :, 0:98]
                        )

                # A2: qkT (f32r) + cosine normalization, per token chunk
                for c0, cs in CHUNKS:
                    qk_ps = qkps.tile([128, 2, 512], F32, tag="qkps", name="qkps")
                    ss_ps = ssps.tile([16, 512], F32, tag="ssps", name="ssps")
                    raw = rawp.tile([128, 4, 512], BF16, tag="raw", name="raw")
                    sq = sqp.tile([128, 4, 512], F32R, tag="sq", name="sq")
                    for ft in range(4):
                        slot = ft % 2
                        for e in range(2):
                            nc.tensor.matmul(
                                qk_ps[:, slot, 0:cs],
                                sb_wqkv[:, e, 128 * ft : 128 * ft + 128],
                                xT[:, e, c0 : c0 + cs],
                                start=(e == 0),
                                stop=(e == 1),
                            )
                        nc.scalar.activation(
                            raw[:, ft, 0:cs], qk_ps[:, slot, 0:cs], AF.Identity,
                            bias=sb_bqkT[:, ft : ft + 1], scale=1.0,
                        )
                        sq_eng = nc.gpsimd if ft < 2 else nc.vector
                        sq_eng.tensor_tensor(
                            out=sq[:, ft, 0:cs], in0=raw[:, ft, 0:cs], in1=raw[:, ft, 0:cs],
                            op=mybir.AluOpType.mult,
                        )
                        nc.tensor.matmul(
                            ss_ps[:, 0:cs], sb_i16[:, ft, :], sq[:, ft, 0:cs],
                            start=(ft == 0), stop=(ft == 3),
                        )
                    rss = nmp.tile([16, 512], F32, tag="rss", name="rss")
                    nc.vector.reciprocal_approx_fast(out=rss[:, 0:cs], in_=ss_ps[:, 0:cs])
                    invn = nmp.tile([16, 512], F32R, tag="invn", name="invn")
                    nc.scalar.activation(invn[:, 0:cs], rss[:, 0:cs], AF.Sqrt, scale=1.0)
                    for ft in range(4):
                        bc_ps = bcps.tile([128, 512], F32, tag="bcps", name="bcps")
                        nc.tensor.matmul(
                            bc_ps[:, 0:cs], sb_bcT[:, ft, :], invn[:, 0:cs],
                            start=True, stop=True,
                        )
                        nc.vector.tensor_tensor(
                            out=qkN[:, ft, c0 : c0 + cs], in0=raw[:, ft, 0:cs],
                            in1=bc_ps[:, 0:cs], op=mybir.AluOpType.mult,
                        )

                # A3: V natural (f32r matmuls from xT as stationary)
                for w in range(NW):
                    for ci, (k0, ks) in enumerate(KC):
                        v_ps = vps.tile([128, 256], F32, tag="vps", name="vps")
                        for e in range(2):
                            nc.tensor.matmul(
                                v_ps[0:ks, :],
                                xT[:, e, w * L + k0 : w * L + k0 + ks],
                                sb_wqkv[:, e, 512:768],
                                start=(e == 0), stop=(e == 1),
                            )
                        nc.vector.scalar_tensor_tensor(
                            out=Vt[ci][:, w, :], in0=v_ps[0:ks, :], scalar=1.0,
                            in1=sb_bv[0:ks, :],
                            op0=mybir.AluOpType.mult, op1=mybir.AluOpType.add,
                        )

            # ---------------- Stage B ----------------
            with ExitStack() as sb_:
                sps = sb_.enter_context(tc.tile_pool(name="sps", bufs=1, space="PSUM"))
                aps = sb_.enter_context(tc.tile_pool(name="aps", bufs=1, space="PSUM"))
                bps = sb_.enter_context(tc.tile_pool(name="bps", bufs=2, space="PSUM"))
                yps = sb_.enter_context(tc.tile_pool(name="yps", bufs=1, space="PSUM"))
                expp = sb_.enter_context(tc.tile_pool(name="expp", bufs=3))
                etp = sb_.enter_context(tc.tile_pool(name="etp", bufs=2))
                invp = sb_.enter_context(tc.tile_pool(name="invp", bufs=2))
                ysb = sb_.enter_context(tc.tile_pool(name="ysb", bufs=2))

                Vv = {ci: Vt[ci][:].rearrange("p w (h d) -> p w h d", h=8) for ci in (0, 1)}

                def emit_s_exp(wp):
                    expT = {}
                    for wi in range(2):
                        w = 2 * wp + wi
                        for ci, (k0, ks) in enumerate(KC):
                            et = expp.tile(
                                [ks, 8, L], BF16, tag=f"exp{wi}_{ci}", name=f"exp{wi}_{ci}"
                            )
                            expT[(wi, ci)] = et
                            for g in range(2):
                                s_ps = sps.tile([128, 4, 512], F32, tag="sps", name="sps")
                                for hp in range(4):
                                    nc.tensor.matmul(
                                        s_ps[0:ks, hp, 0:L],
                                        qkN[32 * hp : 32 * hp + 32, 2 + g,
                                            w * L + k0 : w * L + k0 + ks],
                                        qkN[32 * hp : 32 * hp + 32, g, w * L : w * L + L],
                                        start=True, stop=True,
                                        tile_position=(32 * hp, 0),
                                    )
                                nc.scalar.activation(
                                    et[:, 4 * g : 4 * g + 4, :], s_ps[0:ks, :, 0:L],
                                    AF.Exp, scale=SCALE,
                                )
                    return expT

                def emit_avt_proj(wp, expT):
                    eTs = []
                    for hb in range(2):
                        pA = aps.tile([128, 2, L], F32, tag="pA", name="pA")
                        pB = bps.tile([128, 2, L], F32, tag="pB", name="pB")
                        for wi in range(2):
                            w = 2 * wp + wi
                            for hp in range(4):
                                h = 4 * hb + hp
                                for ci, (k0, ks) in enumerate(KC):
                                    nc.tensor.matmul(
                                        pA[32 * hp : 32 * hp + 32, wi, :],
                                        Vv[ci][:, w, h, :],
                                        expT[(wi, ci)][:, h, :],
                                        start=(ci == 0), stop=(ci == 1),
                                        tile_position=(0, 32 * hp),
                                    )
                                    nc.tensor.matmul(
                                        pB[32 * hp : 32 * hp + 32, wi, :],
                                        sb_ones[0:ks, :],
                                        expT[(wi, ci)][:, h, :],
                                        start=(ci == 0), stop=(ci == 1),
                                        tile_position=(0, 32 * hp),
                                    )
                        inv = invp.tile([128, 2, L], F32, tag="inv", name="inv")
                        nc.vector.reciprocal_approx_fast(out=inv[:], in_=pB[:])
                        eT = etp.tile([128, 2, L], F32R, tag=f"eT{hb}", name=f"eT{hb}")
                        eTs.append(eT)
                        with nc.allow_low_precision(reason="attn out f32r for proj"):
                            nc.vector.tensor_tensor(
                                out=eT[:], in0=pA[:], in1=inv[:], op=mybir.AluOpType.mult
                            )
                    for wi in range(2):
                        w = 2 * wp + wi
                        y_ps = yps.tile([98, 2, 256], F32, tag="yps", name="yps")
                        for th in range(2):
                            for hb in range(2):
                                nc.tensor.matmul(
                                    y_ps[:, th, :],
                                    eTs[hb][:, wi, 98 * th : 98 * th + 98],
                                    sb_wproj[:, hb, :],
                                    start=(hb == 0), stop=(hb == 1),
                                )
                        y_sb = ysb.tile([98, 2, 256], F32, tag="ysb", name="ysb")
                        nc.vector.scalar_tensor_tensor(
                            out=y_sb[:], in0=y_ps[:], scalar=1.0, in1=sb_bp[0:98, :, :],
                            op0=mybir.AluOpType.mult, op1=mybir.AluOpType.add,
                        )
                        nc.sync.dma_start(
                            out=out[w, :, :].rearrange("(th p) e -> p th e", th=2),
                            in_=y_sb[:],
                        )

                prev = None
                for wp in range(NW // 2):
                    cur = emit_s_exp(wp)
                    if prev is not None:
                        emit_avt_proj(wp - 1, prev)
                    prev = cur
                emit_avt_proj(NW // 2 - 1, prev)

    nc.finalize()
    return nc


_NC = None


def _get_nc():
    global _NC
    if _NC is None:
        _NC = _build()
    return _NC


def _consts():
    import ml_dtypes

    p = np.arange(128)
    ind16 = np.zeros((128, 4, 16), np.float32)
    bcT = np.zeros((16, 4, 128), np.float32)
    for ft in range(4):
        ind16[p, ft, 4 * ft + p // 32] = 1.0
        bcT[4 * ft + p // 32, ft, p] = 1.0
    return {
        "ident": np.eye(128, dtype=np.float32),
        "ind16": ind16,
        "bcT": bcT,
        "onesb": np.ones((128, 32), ml_dtypes.bfloat16),
    }


def kernel(x, w_qkv, b_qkv, w_proj, b_proj, _trace=False):
    x = np.ascontiguousarray(np.asarray(x, np.float32))
    w_qkv = np.asarray(w_qkv, np.float32)
    b_qkv = np.asarray(b_qkv, np.float32)
    w_proj = np.asarray(w_proj, np.float32)
    b_proj = np.asarray(b_proj, np.float32)

    B, H, W, _ = x.shape
    xw = (
        x.reshape(NCORES, 2, 4, 14, 4, 14, E)
        .transpose(0, 1, 2, 4, 3, 5, 6)
        .reshape(NCORES, NW, L, E)
    )
    base = {
        "w_qkv": np.ascontiguousarray(w_qkv.reshape(2, 128, 768)),
        "bqkT": np.ascontiguousarray(b_qkv[:512].reshape(4, 128).T),
        "w_proj": np.ascontiguousarray(w_proj.reshape(2, 128, 256)),
        "bv_bc": np.broadcast_to(b_qkv[512:768], (128, 256)).copy(),
        "bp_bc": np.broadcast_to(b_proj, (128, 2, 256)).copy(),
        **_consts(),
    }
    in_maps = [dict(base, x=np.ascontiguousarray(xw[c])) for c in range(NCORES)]

    nc = _get_nc()
    br = run_bass_kernel_spmd(nc, in_maps, list(range(NCORES)), trace=_trace)
    outs = np.stack([br.results[c]["out"] for c in range(NCORES)])
    y = (
        outs.reshape(NCORES, 2, 4, 4, 14, 14, E)
        .transpose(0, 1, 2, 4, 3, 5, 6)
        .reshape(B, H, W, E)
    )
    if _trace:
        return y, br.exec_time_ns
    return y


# revision 15
# speedup vs baseline: 1.1261x; 1.0170x over previous
"""AngularAttention (windowed cosine attention) Trainium2 kernel, 8-core data-parallel.

Per core = 2 images = 32 windows x 196 tokens. See stage comments inline.
"""
import sys

sys.path.insert(0, "/opt/trn_rl_repo")

import numpy as np

import concourse.bacc as bacc
import concourse.mybir as mybir
import concourse.tile as tile
from concourse.bass_utils import run_bass_kernel_spmd

F32 = mybir.dt.float32
F32R = mybir.dt.float32r
BF16 = mybir.dt.bfloat16
AF = mybir.ActivationFunctionType

NCORES = 8
NW = 32
L = 196
T = NW * L
E = 256
SCALE = 10.0

CHUNKS = [(i * 512, 512) for i in range(12)] + [(6144, 128)]
KC = [(0, 128), (128, 68)]


def _build():
    nc = bacc.Bacc(None)
    x = nc.declare_dram_parameter("x", [NW, L, E], F32, isOutput=False)
    w_qkv = nc.declare_dram_parameter("w_qkv", [2, 128, 768], F32R, isOutput=False)
    bqkT = nc.declare_dram_parameter("bqkT", [128, 4], F32, isOutput=False)
    w_proj = nc.declare_dram_parameter("w_proj", [2, 128, 256], F32R, isOutput=False)
    bv_bc = nc.declare_dram_parameter("bv_bc", [128, 256], F32, isOutput=False)
    bp_bc = nc.declare_dram_parameter("bp_bc", [128, 2, 256], F32, isOutput=False)
    ident = nc.declare_dram_parameter("ident", [128, 128], F32, isOutput=False)
    ind16 = nc.declare_dram_parameter("ind16", [128, 4, 128], F32R, isOutput=False)
    bcT = nc.declare_dram_parameter("bcT", [128, 4, 128], F32R, isOutput=False)
    onesb = nc.declare_dram_parameter("onesb", [128, 32], BF16, isOutput=False)
    out = nc.declare_dram_parameter("out", [NW, L, E], F32, isOutput=True)

    with tile.TileContext(nc) as tc:
        from contextlib import ExitStack

        with ExitStack() as root:
            const = root.enter_context(tc.tile_pool(name="const", bufs=1))
            # f32 arena: [0:4] bqkT, [4:132] ident, [132:388] bv, [388:900] bp
            sb_m = const.tile([128, 900], F32)
            nc.sync.dma_start(out=sb_m[:, 0:4], in_=bqkT[:, :])
            nc.sync.dma_start(out=sb_m[:, 4:132], in_=ident[:, :])
            nc.sync.dma_start(out=sb_m[:, 132:388], in_=bv_bc[:, :])
            nc.sync.dma_start(
                out=sb_m[:, 388:900], in_=bp_bc[:, :, :].rearrange("p a b -> p (a b)")
            )
            sb_bqkT = sb_m[:, 0:4]
            sb_id = sb_m[:, 4:132]
            sb_bv = sb_m[:, 132:388]
            sb_bp = sb_m[:, 388:900].rearrange("p (a b) -> p a b", a=2)
            # f32r arena: [0:1536] w_qkv (2,768), [1536:1600] ind16 (4,16), [1600:2112] w_proj (2,256)
            sb_r = const.tile([128, 2560], F32R)
            nc.sync.dma_start(
                out=sb_r[:, 0:1536].rearrange("p (c f) -> p c f", c=2),
                in_=w_qkv[:, :, :].rearrange("c p f -> p c f"),
            )
            nc.sync.dma_start(
                out=sb_r[:, 1536:2048], in_=ind16[:, :, :].rearrange("p a b -> p (a b)")
            )
            nc.sync.dma_start(
                out=sb_r[:, 2048:2560].rearrange("p (c f) -> p c f", c=2),
                in_=w_proj[:, :, :].rearrange("c p f -> p c f"),
            )
            sb_wqkv = sb_r[:, 0:1536].rearrange("p (c f) -> p c f", c=2)
            sb_i16 = sb_r[:, 1536:2048].rearrange("p (a b) -> p a b", a=4)
            sb_wproj = sb_r[:, 2048:2560].rearrange("p (c f) -> p c f", c=2)

            sb_bcT = const.tile([128, 4, 128], F32R)
            nc.sync.dma_start(out=sb_bcT[:], in_=bcT[:, :, :])
            sb_ones = const.tile([128, 32], BF16)
            nc.sync.dma_start(out=sb_ones[:], in_=onesb[:, :])

            big = root.enter_context(tc.tile_pool(name="big", bufs=1))
            qkN = big.tile([128, 4, T], BF16)        # q heads 0-3 | q 4-7 | k 0-3 | k 4-7
            V0 = big.tile([128, NW, 256], BF16)      # k-chunk0 of each window, [tok, (head d)]
            V1 = big.tile([68, NW, 256], BF16)
            Vt = {0: V0, 1: V1}

            # ---------------- Stage A ----------------
            with ExitStack() as sa:
                xin = sa.enter_context(tc.tile_pool(name="xin", bufs=3))
                qkps = sa.enter_context(tc.tile_pool(name="qkps", bufs=2, space="PSUM"))
                ssps = sa.enter_context(tc.tile_pool(name="ssps", bufs=1, space="PSUM"))
                bcps = sa.enter_context(tc.tile_pool(name="bcps", bufs=2, space="PSUM"))
                vps = sa.enter_context(tc.tile_pool(name="vps", bufs=1, space="PSUM"))
                xtp = sa.enter_context(tc.tile_pool(name="xtp", bufs=1))
                rawp = sa.enter_context(tc.tile_pool(name="rawp", bufs=3))
                sqp = sa.enter_context(tc.tile_pool(name="sqp", bufs=2))
                nmp = sa.enter_context(tc.tile_pool(name="nmp", bufs=2))

                xT = xtp.tile([128, 2, T], F32R)

                # A1: load x + PE-transpose, 4 halves (98 tok) per psum batch
                for b0 in range(0, NW * 2, 4):
                    ptr = [
                        qkps.tile([128, 4, 128], F32, tag="qkps", name=f"tr{e}")
                        for e in range(2)
                    ]
                    for s in range(4):
                        hf = b0 + s
                        w, th = hf // 2, hf % 2
                        xt_in = xin.tile([98, 256], F32)
                        nc.sync.dma_start(out=xt_in[:], in_=x[w, 98 * th : 98 * th + 98, :])
                        for e in range(2):
                            nc.tensor.transpose(
                                ptr[e][:, s, 0:98],
                                xt_in[:, 128 * e : 128 * e + 128],
                                sb_id[0:98, 0:98],
                            )
                    for e in range(2):
                        nc.vector.tensor_copy(
                            xT[:, e, 98 * b0 : 98 * (b0 + 4)], ptr[e][:, :, 0:98]
                        )

                # A2: qkT (f32r) + cosine normalization, per token chunk
                for c0, cs in CHUNKS:
                    qk_ps = qkps.tile([128, 2, 512], F32, tag="qkps", name="qkps")
                    ss_ps = ssps.tile([128, 512], F32, tag="ssps", name="ssps")
                    raw = rawp.tile([128, 4, 512], BF16, tag="raw", name="raw")
                    sq = sqp.tile([128, 4, 512], F32R, tag="sq", name="sq")
                    for ft in range(4):
                        slot = ft % 2
                        for e in range(2):
                            nc.tensor.matmul(
                                qk_ps[:, slot, 0:cs],
                                sb_wqkv[:, e, 128 * ft : 128 * ft + 128],
                                xT[:, e, c0 : c0 + cs],
                                start=(e == 0),
                                stop=(e == 1),
                            )
                        nc.scalar.activation(
                            raw[:, ft, 0:cs], qk_ps[:, slot, 0:cs], AF.Identity,
                            bias=sb_bqkT[:, ft : ft + 1], scale=1.0,
                        )
                        sq_eng = nc.gpsimd if ft < 2 else nc.vector
                        sq_eng.tensor_tensor(
                            out=sq[:, ft, 0:cs], in0=raw[:, ft, 0:cs], in1=raw[:, ft, 0:cs],
                            op=mybir.AluOpType.mult,
                        )
                        nc.tensor.matmul(
                            ss_ps[:, 0:cs], sb_i16[:, ft, :], sq[:, ft, 0:cs],
                            start=(ft == 0), stop=(ft == 3),
                        )
                    rss = nmp.tile([128, 512], F32, tag="rss", name="rss")
                    nc.vector.reciprocal_approx_fast(out=rss[:, 0:cs], in_=ss_ps[:, 0:cs])
                    invn = nmp.tile([128, 512], F32R, tag="invn", name="invn")
                    nc.scalar.activation(invn[:, 0:cs], rss[:, 0:cs], AF.Sqrt, scale=1.0)
                    for ft in range(4):
                        bc_ps = bcps.tile([128, 512], F32, tag="bcps", name="bcps")
                        nc.tensor.matmul(
                            bc_ps[:, 0:cs], sb_bcT[:, ft, :], invn[:, 0:cs],
                            start=True, stop=True,
                        )
                        nc.vector.tensor_tensor(
                            out=qkN[:, ft, c0 : c0 + cs], in0=raw[:, ft, 0:cs],
                            in1=bc_ps[:, 0:cs], op=mybir.AluOpType.mult,
                        )

                # A3: V natural (f32r matmuls from xT as stationary)
                for w in range(NW):
                    for ci, (k0, ks) in enumerate(KC):
                        v_ps = vps.tile([128, 256], F32, tag="vps", name="vps")
                        for e in range(2):
                            nc.tensor.matmul(
                                v_ps[0:ks, :],
                                xT[:, e, w * L + k0 : w * L + k0 + ks],
                                sb_wqkv[:, e, 512:768],
                                start=(e == 0), stop=(e == 1),
                            )
                        nc.vector.scalar_tensor_tensor(
                            out=Vt[ci][:, w, :], in0=v_ps[0:ks, :], scalar=1.0,
                            in1=sb_bv[0:ks, :],
                            op0=mybir.AluOpType.mult, op1=mybir.AluOpType.add,
                        )

            # ---------------- Stage B ----------------
            with ExitStack() as sb_:
                sps = sb_.enter_context(tc.tile_pool(name="sps", bufs=1, space="PSUM"))
                aps = sb_.enter_context(tc.tile_pool(name="aps", bufs=1, space="PSUM"))
                bps = sb_.enter_context(tc.tile_pool(name="bps", bufs=2, space="PSUM"))
                yps = sb_.enter_context(tc.tile_pool(name="yps", bufs=1, space="PSUM"))
                expp = sb_.enter_context(tc.tile_pool(name="expp", bufs=3))
                etp = sb_.enter_context(tc.tile_pool(name="etp", bufs=2))
                invp = sb_.enter_context(tc.tile_pool(name="invp", bufs=2))
                ysb = sb_.enter_context(tc.tile_pool(name="ysb", bufs=2))

                Vv = {ci: Vt[ci][:].rearrange("p w (h d) -> p w h d", h=8) for ci in (0, 1)}

                def emit_s_exp(wp):
                    expT = {}
                    for wi in range(2):
                        w = 2 * wp + wi
                        for ci, (k0, ks) in enumerate(KC):
                            et = expp.tile(
                                [ks, 8, L], BF16, tag=f"exp{wi}_{ci}", name=f"exp{wi}_{ci}"
                            )
                            expT[(wi, ci)] = et
                            for g in range(2):
                                s_ps = sps.tile([128, 4, 512], F32, tag="sps", name="sps")
                                for hp in range(4):
                                    nc.tensor.matmul(
                                        s_ps[0:ks, hp, 0:L],
                                        qkN[32 * hp : 32 * hp + 32, 2 + g,
                                            w * L + k0 : w * L + k0 + ks],
                                        qkN[32 * hp : 32 * hp + 32, g, w * L : w * L + L],
                                        start=True, stop=True,
                                        tile_position=(32 * hp, 0),
                                    )
                                nc.scalar.activation(
                                    et[:, 4 * g : 4 * g + 4, :], s_ps[0:ks, :, 0:L],
                                    AF.Exp, scale=SCALE,
                                )
                    return expT

                def emit_avt_proj(wp, expT):
                    eTs = []
                    for hb in range(2):
                        pA = aps.tile([128, 2, L], F32, tag="pA", name="pA")
                        pB = bps.tile([128, 2, L], F32, tag="pB", name="pB")
                        for wi in range(2):
                            w = 2 * wp + wi
                            for hp in range(4):
                                h = 4 * hb + hp
                                for ci, (k0, ks) in enumerate(KC):
                                    nc.tensor.matmul(
                                        pA[32 * hp : 32 * hp + 32, wi, :],
                                        Vv[ci][:, w, h, :],
                                        expT[(wi, ci)][:, h, :],
                                        start=(ci == 0), stop=(ci == 1),
                                        tile_position=(0, 32 * hp),
                                    )
                                    nc.tensor.matmul(
                                        pB[32 * hp : 32 * hp + 32, wi, :],
                                        sb_ones[0:ks, :],
                                        expT[(wi, ci)][:, h, :],
                                        start=(ci == 0), stop=(ci == 1),
                                        tile_position=(0, 32 * hp),
                                    )
                        inv = invp.tile([128, 2, L], F32, tag="inv", name="inv")
                        nc.vector.reciprocal_approx_fast(out=inv[:], in_=pB[:])
                        eT = etp.tile([128, 2, L], F32R, tag=f"eT{hb}", name=f"eT{hb}")
                        eTs.append(eT)
                        with nc.allow_low_precision(reason="attn out f32r for proj"):
                            nc.vector.tensor_tensor(
                                out=eT[:], in0=pA[:], in1=inv[:], op=mybir.AluOpType.mult
                            )
                    for wi in range(2):
                        w = 2 * wp + wi
                        y_ps = yps.tile([98, 2, 256], F32, tag="yps", name="yps")
                        for th in range(2):
                            for hb in range(2):
                                nc.tensor.matmul(
                                    y_ps[:, th, :],
                                    eTs[hb][:, wi, 98 * th : 98 * th + 98],
                                    sb_wproj[:, hb, :],
                                    start=(hb == 0), stop=(hb == 1),
                                )
                        y_sb = ysb.tile([98, 2, 256], F32, tag="ysb", name="ysb")
                        nc.vector.scalar_tensor_tensor(
                            out=y_sb[:], in0=y_ps[:], scalar=1.0, in1=sb_bp[0:98, :, :],
                            op0=mybir.AluOpType.mult, op1=mybir.AluOpType.add,
                        )
                        nc.sync.dma_start(
                            out=out[w, :, :].rearrange("(th p) e -> p th e", th=2),
                            in_=y_sb[:],
                        )

                prev = None
                for wp in range(NW // 2):
                    cur = emit_s_exp(wp)
                    if prev is not None:
                        emit_avt_proj(wp - 1, prev)
                    prev = cur
                emit_avt_proj(NW // 2 - 1, prev)

    nc.finalize()
    return nc


_NC = None


def _get_nc():
    global _NC
    if _NC is None:
        _NC = _build()
    return _NC


def _consts():
    import ml_dtypes

    p = np.arange(128)
    # sumsq lhsT, padded to M=128: cols 16.. get 1e-6 so padded sumsq rows stay
    # positive (recip/sqrt-safe); bcast lhsT padded to K=128 with zero rows.
    ind16 = np.zeros((128, 4, 128), np.float32)
    ind16[:, :, 16:] = 1e-6
    bcT = np.zeros((128, 4, 128), np.float32)
    for ft in range(4):
        ind16[p, ft, 4 * ft + p // 32] = 1.0
        bcT[4 * ft + p // 32, ft, p] = 1.0
    return {
        "ident": np.eye(128, dtype=np.float32),
        "ind16": ind16,
        "bcT": bcT,
        "onesb": np.ones((128, 32), ml_dtypes.bfloat16),
    }


def kernel(x, w_qkv, b_qkv, w_proj, b_proj, _trace=False):
    x = np.ascontiguousarray(np.asarray(x, np.float32))
    w_qkv = np.asarray(w_qkv, np.float32)
    b_qkv = np.asarray(b_qkv, np.float32)
    w_proj = np.asarray(w_proj, np.float32)
    b_proj = np.asarray(b_proj, np.float32)

    B, H, W, _ = x.shape
    xw = (
        x.reshape(NCORES, 2, 4, 14, 4, 14, E)
        .transpose(0, 1, 2, 4, 3, 5, 6)
        .reshape(NCORES, NW, L, E)
    )
    base = {
        "w_qkv": np.ascontiguousarray(w_qkv.reshape(2, 128, 768)),
        "bqkT": np.ascontiguousarray(b_qkv[:512].reshape(4, 128).T),
        "w_proj": np.ascontiguousarray(w_proj.reshape(2, 128, 256)),
        "bv_bc": np.broadcast_to(b_qkv[512:768], (128, 256)).copy(),
        "bp_bc": np.broadcast_to(b_proj, (128, 2, 256)).copy(),
        **_consts(),
    }
    in_maps = [dict(base, x=np.ascontiguousarray(xw[c])) for c in range(NCORES)]

    nc = _get_nc()
    br = run_bass_kernel_spmd(nc, in_maps, list(range(NCORES)), trace=_trace)
    outs = np.stack([br.results[c]["out"] for c in range(NCORES)])
    y = (
        outs.reshape(NCORES, 2, 4, 4, 14, 14, E)
        .transpose(0, 1, 2, 4, 3, 5, 6)
        .reshape(B, H, W, E)
    )
    if _trace:
        return y, br.exec_time_ns
    return y


# revision 16
# speedup vs baseline: 1.2024x; 1.0678x over previous
"""AngularAttention (windowed cosine attention) Trainium2 kernel, 8-core data-parallel.

Per core = 2 images = 32 windows x 196 tokens. See stage comments inline.
"""
import sys

sys.path.insert(0, "/opt/trn_rl_repo")

import numpy as np

import concourse.bacc as bacc
import concourse.mybir as mybir
import concourse.tile as tile
from concourse.bass_utils import run_bass_kernel_spmd

F32 = mybir.dt.float32
F32R = mybir.dt.float32r
BF16 = mybir.dt.bfloat16
AF = mybir.ActivationFunctionType

NCORES = 8
NW = 32
L = 196
T = NW * L
E = 256
SCALE = 10.0

CHUNKS = [(i * 512, 512) for i in range(12)] + [(6144, 128)]
KC = [(0, 128), (128, 68)]


def _build():
    nc = bacc.Bacc(None)
    x = nc.declare_dram_parameter("x", [NW, L, E], F32, isOutput=False)
    w_qkv = nc.declare_dram_parameter("w_qkv", [2, 128, 768], F32R, isOutput=False)
    bqkT = nc.declare_dram_parameter("bqkT", [128, 4], F32, isOutput=False)
    w_proj = nc.declare_dram_parameter("w_proj", [2, 128, 256], F32R, isOutput=False)
    bv_bc = nc.declare_dram_parameter("bv_bc", [128, 256], F32, isOutput=False)
    bp_bc = nc.declare_dram_parameter("bp_bc", [128, 2, 256], F32, isOutput=False)
    ident = nc.declare_dram_parameter("ident", [128, 128], F32, isOutput=False)
    ind16 = nc.declare_dram_parameter("ind16", [128, 4, 128], F32R, isOutput=False)
    bcT = nc.declare_dram_parameter("bcT", [128, 4, 128], F32R, isOutput=False)
    onesb = nc.declare_dram_parameter("onesb", [128, 32], BF16, isOutput=False)
    out = nc.declare_dram_parameter("out", [NW, L, E], F32, isOutput=True)

    with tile.TileContext(nc) as tc:
        from contextlib import ExitStack

        with ExitStack() as root:
            const = root.enter_context(tc.tile_pool(name="const", bufs=1))
            # f32 arena: [0:4] bqkT, [4:132] ident, [132:388] bv, [388:900] bp
            sb_m = const.tile([128, 900], F32)
            nc.sync.dma_start(out=sb_m[:, 0:4], in_=bqkT[:, :])
            nc.sync.dma_start(out=sb_m[:, 4:132], in_=ident[:, :])
            nc.sync.dma_start(out=sb_m[:, 132:388], in_=bv_bc[:, :])
            nc.sync.dma_start(
                out=sb_m[:, 388:900], in_=bp_bc[:, :, :].rearrange("p a b -> p (a b)")
            )
            sb_bqkT = sb_m[:, 0:4]
            sb_id = sb_m[:, 4:132]
            sb_bv = sb_m[:, 132:388]
            sb_bp = sb_m[:, 388:900].rearrange("p (a b) -> p a b", a=2)
            # f32r arena: [0:1536] w_qkv (2,768), [1536:1600] ind16 (4,16), [1600:2112] w_proj (2,256)
            sb_r = const.tile([128, 2560], F32R)
            nc.sync.dma_start(
                out=sb_r[:, 0:1536].rearrange("p (c f) -> p c f", c=2),
                in_=w_qkv[:, :, :].rearrange("c p f -> p c f"),
            )
            nc.sync.dma_start(
                out=sb_r[:, 1536:2048], in_=ind16[:, :, :].rearrange("p a b -> p (a b)")
            )
            nc.sync.dma_start(
                out=sb_r[:, 2048:2560].rearrange("p (c f) -> p c f", c=2),
                in_=w_proj[:, :, :].rearrange("c p f -> p c f"),
            )
            sb_wqkv = sb_r[:, 0:1536].rearrange("p (c f) -> p c f", c=2)
            sb_i16 = sb_r[:, 1536:2048].rearrange("p (a b) -> p a b", a=4)
            sb_wproj = sb_r[:, 2048:2560].rearrange("p (c f) -> p c f", c=2)

            sb_bcT = const.tile([128, 4, 128], F32R)
            nc.sync.dma_start(out=sb_bcT[:], in_=bcT[:, :, :])
            sb_ones = const.tile([128, 32], BF16)
            nc.sync.dma_start(out=sb_ones[:], in_=onesb[:, :])

            big = root.enter_context(tc.tile_pool(name="big", bufs=1))
            qkN = big.tile([128, 4, T], BF16)        # q heads 0-3 | q 4-7 | k 0-3 | k 4-7
            V0 = big.tile([128, NW, 256], BF16)      # k-chunk0 of each window, [tok, (head d)]
            V1 = big.tile([68, NW, 256], BF16)
            Vt = {0: V0, 1: V1}

            # ---------------- Stage A ----------------
            with ExitStack() as sa:
                xin = sa.enter_context(tc.tile_pool(name="xin", bufs=3))
                qkps = sa.enter_context(tc.tile_pool(name="qkps", bufs=2, space="PSUM"))
                ssps = sa.enter_context(tc.tile_pool(name="ssps", bufs=1, space="PSUM"))
                bcps = sa.enter_context(tc.tile_pool(name="bcps", bufs=2, space="PSUM"))
                vps = sa.enter_context(tc.tile_pool(name="vps", bufs=1, space="PSUM"))
                xtp = sa.enter_context(tc.tile_pool(name="xtp", bufs=1))
                rawp = sa.enter_context(tc.tile_pool(name="rawp", bufs=3))
                sqp = sa.enter_context(tc.tile_pool(name="sqp", bufs=2))
                nmp = sa.enter_context(tc.tile_pool(name="nmp", bufs=2))

                xT = xtp.tile([128, 2, T], F32R)

                # A1: load x + PE-transpose, 4 halves (98 tok) per psum batch
                for b0 in range(0, NW * 2, 4):
                    ptr = [
                        qkps.tile([128, 4, 128], F32, tag="qkps", name=f"tr{e}")
                        for e in range(2)
                    ]
                    for s in range(4):
                        hf = b0 + s
                        w, th = hf // 2, hf % 2
                        xt_in = xin.tile([98, 256], F32)
                        nc.sync.dma_start(out=xt_in[:], in_=x[w, 98 * th : 98 * th + 98, :])
                        for e in range(2):
                            nc.tensor.transpose(
                                ptr[e][:, s, 0:98],
                                xt_in[:, 128 * e : 128 * e + 128],
                                sb_id[0:98, 0:98],
                            )
                    for e in range(2):
                        nc.vector.tensor_copy(
                            xT[:, e, 98 * b0 : 98 * (b0 + 4)], ptr[e][:, :, 0:98]
                        )

                # A2: qkT (f32r) + cosine normalization, per token chunk
                for c0, cs in CHUNKS:
                    qk_ps = qkps.tile([128, 2, 512], F32, tag="qkps", name="qkps")
                    ss_ps = ssps.tile([128, 512], F32, tag="ssps", name="ssps")
                    raw = rawp.tile([128, 4, 512], BF16, tag="raw", name="raw")
                    sq = sqp.tile([128, 4, 512], F32R, tag="sq", name="sq")
                    for ft in range(4):
                        slot = ft % 2
                        for e in range(2):
                            nc.tensor.matmul(
                                qk_ps[:, slot, 0:cs],
                                sb_wqkv[:, e, 128 * ft : 128 * ft + 128],
                                xT[:, e, c0 : c0 + cs],
                                start=(e == 0),
                                stop=(e == 1),
                            )
                        nc.scalar.activation(
                            raw[:, ft, 0:cs], qk_ps[:, slot, 0:cs], AF.Identity,
                            bias=sb_bqkT[:, ft : ft + 1], scale=1.0,
                        )
                        sq_eng = nc.gpsimd if ft < 2 else nc.vector
                        sq_eng.tensor_tensor(
                            out=sq[:, ft, 0:cs], in0=raw[:, ft, 0:cs], in1=raw[:, ft, 0:cs],
                            op=mybir.AluOpType.mult,
                        )
                        nc.tensor.matmul(
                            ss_ps[:, 0:cs], sb_i16[:, ft, :], sq[:, ft, 0:cs],
                            start=(ft == 0), stop=(ft == 3),
                        )
                    rss = nmp.tile([128, 512], F32, tag="rss", name="rss")
                    nc.vector.reciprocal_approx_fast(out=rss[:, 0:cs], in_=ss_ps[:, 0:cs])
                    invn = nmp.tile([128, 512], F32R, tag="invn", name="invn")
                    nc.scalar.activation(invn[:, 0:cs], rss[:, 0:cs], AF.Sqrt, scale=1.0)
                    for ft in range(4):
                        bc_ps = bcps.tile([128, 512], F32, tag="bcps", name="bcps")
                        nc.tensor.matmul(
                            bc_ps[:, 0:cs], sb_bcT[:, ft, :], invn[:, 0:cs],
                            start=True, stop=True,
                        )
                        nc.vector.tensor_tensor(
                            out=qkN[:, ft, c0 : c0 + cs], in0=raw[:, ft, 0:cs],
                            in1=bc_ps[:, 0:cs], op=mybir.AluOpType.mult,
                        )

                # A3: V natural (f32r matmuls from xT as stationary)
                for w in range(NW):
                    for ci, (k0, ks) in enumerate(KC):
                        v_ps = vps.tile([128, 256], F32, tag="vps", name="vps")
                        for e in range(2):
                            nc.tensor.matmul(
                                v_ps[0:ks, :],
                                xT[:, e, w * L + k0 : w * L + k0 + ks],
                                sb_wqkv[:, e, 512:768],
                                start=(e == 0), stop=(e == 1),
                            )
                        nc.vector.scalar_tensor_tensor(
                            out=Vt[ci][:, w, :], in0=v_ps[0:ks, :], scalar=1.0,
                            in1=sb_bv[0:ks, :],
                            op0=mybir.AluOpType.mult, op1=mybir.AluOpType.add,
                        )

            # ---------------- Stage B ----------------
            # Quarter passes: B1 = S+exp for 8 windows (PSUM fully dedicated to
            # double-buffered S groups), then B2 = AVT+proj (deep-pipelined).
            etp_ctx = root.enter_context(tc.tile_pool(name="etp", bufs=2))
            invp = root.enter_context(tc.tile_pool(name="invp", bufs=2))
            ysb = root.enter_context(tc.tile_pool(name="ysb", bufs=2))
            etp = etp_ctx
            QW = 8
            Vv = {ci: Vt[ci][:].rearrange("p w (h d) -> p w h d", h=8) for ci in (0, 1)}
            for q in range(NW // QW):
                w0 = q * QW
                with ExitStack() as sq_:
                    expq = sq_.enter_context(tc.tile_pool(name=f"expq{q}", bufs=1))
                    EQ = {
                        ci: expq.tile([ks, QW, 8, L], BF16, tag=f"eq{ci}", name=f"eq{ci}")
                        for ci, (k0, ks) in enumerate(KC)
                    }
                    # ---- B1: S + exp ----
                    with ExitStack() as sb1:
                        sps = sb1.enter_context(tc.tile_pool(name="sps", bufs=2, space="PSUM"))
                        for wl in range(QW):
                            w = w0 + wl
                            for ci, (k0, ks) in enumerate(KC):
                                for g in range(2):
                                    s_ps = sps.tile([128, 4, 512], F32, tag="sps", name="sps")
                                    for hp in range(4):
                                        nc.tensor.matmul(
                                            s_ps[0:ks, hp, 0:L],
                                            qkN[32 * hp : 32 * hp + 32, 2 + g,
                                                w * L + k0 : w * L + k0 + ks],
                                            qkN[32 * hp : 32 * hp + 32, g, w * L : w * L + L],
                                            start=True, stop=True,
                                            tile_position=(32 * hp, 0),
                                        )
                                    nc.scalar.activation(
                                        EQ[ci][:, wl, 4 * g : 4 * g + 4, :],
                                        s_ps[0:ks, :, 0:L],
                                        AF.Exp, scale=SCALE,
                                    )
                    # ---- B2: AVT + proj ----
                    with ExitStack() as sb2:
                        aps = sb2.enter_context(tc.tile_pool(name="aps", bufs=2, space="PSUM"))
                        bps = sb2.enter_context(tc.tile_pool(name="bps", bufs=2, space="PSUM"))
                        yps = sb2.enter_context(tc.tile_pool(name="yps", bufs=2, space="PSUM"))
                        for wp in range(QW // 2):
                            eTs = []
                            for hb in range(2):
                                pA = aps.tile([128, 2, L], F32, tag="pA", name="pA")
                                pB = bps.tile([128, 2, L], F32, tag="pB", name="pB")
                                for wi in range(2):
                                    wl = 2 * wp + wi
                                    w = w0 + wl
                                    for hp in range(4):
                                        h = 4 * hb + hp
                                        for ci, (k0, ks) in enumerate(KC):
                                            nc.tensor.matmul(
                                                pA[32 * hp : 32 * hp + 32, wi, :],
                                                Vv[ci][:, w, h, :],
                                                EQ[ci][:, wl, h, :],
                                                start=(ci == 0), stop=(ci == 1),
                                                tile_position=(0, 32 * hp),
                                            )
                                            nc.tensor.matmul(
                                                pB[32 * hp : 32 * hp + 32, wi, :],
                                                sb_ones[0:ks, :],
                                                EQ[ci][:, wl, h, :],
                                                start=(ci == 0), stop=(ci == 1),
                                                tile_position=(0, 32 * hp),
                                            )
                                inv = invp.tile([128, 2, L], F32, tag="inv", name="inv")
                                nc.vector.reciprocal_approx_fast(out=inv[:], in_=pB[:])
                                eT = etp.tile([128, 2, L], F32R, tag=f"eT{hb}", name=f"eT{hb}")
                                eTs.append(eT)
                                with nc.allow_low_precision(reason="attn out f32r for proj"):
                                    nc.vector.tensor_tensor(
                                        out=eT[:], in0=pA[:], in1=inv[:], op=mybir.AluOpType.mult
                                    )
                            for wi in range(2):
                                w = w0 + 2 * wp + wi
                                y_ps = yps.tile([98, 2, 256], F32, tag="yps", name="yps")
                                for th in range(2):
                                    for hb in range(2):
                                        nc.tensor.matmul(
                                            y_ps[:, th, :],
                                            eTs[hb][:, wi, 98 * th : 98 * th + 98],
                                            sb_wproj[:, hb, :],
                                            start=(hb == 0), stop=(hb == 1),
                                        )
                                y_sb = ysb.tile([98, 2, 256], F32, tag="ysb", name="ysb")
                                nc.vector.scalar_tensor_tensor(
                                    out=y_sb[:], in0=y_ps[:], scalar=1.0, in1=sb_bp[0:98, :, :],
                                    op0=mybir.AluOpType.mult, op1=mybir.AluOpType.add,
                                )
                                nc.sync.dma_start(
                                    out=out[w, :, :].rearrange("(th p) e -> p th e", th=2),
                                    in_=y_sb[:],
                                )

    nc.finalize()
    return nc


_NC = None


def _get_nc():
    global _NC
    if _NC is None:
        _NC = _build()
    return _NC


def _consts():
    import ml_dtypes

    p = np.arange(128)
    # sumsq lhsT, padded to M=128: cols 16.. get 1e-6 so padded sumsq rows stay
    # positive (recip/sqrt-safe); bcast lhsT padded to K=128 with zero rows.
    ind16 = np.zeros((128, 4, 128), np.float32)
    ind16[:, :, 16:] = 1e-6
    bcT = np.zeros((128, 4, 128), np.float32)
    for ft in range(4):
        ind16[p, ft, 4 * ft + p // 32] = 1.0
        bcT[4 * ft + p // 32, ft, p] = 1.0
    return {
        "ident": np.eye(128, dtype=np.float32),
        "ind16": ind16,
        "bcT": bcT,
        "onesb": np.ones((128, 32), ml_dtypes.bfloat16),
    }


def kernel(x, w_qkv, b_qkv, w_proj, b_proj, _trace=False):
    x = np.ascontiguousarray(np.asarray(x, np.float32))
    w_qkv = np.asarray(w_qkv, np.float32)
    b_qkv = np.asarray(b_qkv, np.float32)
    w_proj = np.asarray(w_proj, np.float32)
    b_proj = np.asarray(b_proj, np.float32)

    B, H, W, _ = x.shape
    xw = (
        x.reshape(NCORES, 2, 4, 14, 4, 14, E)
        .transpose(0, 1, 2, 4, 3, 5, 6)
        .reshape(NCORES, NW, L, E)
    )
    base = {
        "w_qkv": np.ascontiguousarray(w_qkv.reshape(2, 128, 768)),
        "bqkT": np.ascontiguousarray(b_qkv[:512].reshape(4, 128).T),
        "w_proj": np.ascontiguousarray(w_proj.reshape(2, 128, 256)),
        "bv_bc": np.broadcast_to(b_qkv[512:768], (128, 256)).copy(),
        "bp_bc": np.broadcast_to(b_proj, (128, 2, 256)).copy(),
        **_consts(),
    }
    in_maps = [dict(base, x=np.ascontiguousarray(xw[c])) for c in range(NCORES)]

    nc = _get_nc()
    br = run_bass_kernel_spmd(nc, in_maps, list(range(NCORES)), trace=_trace)
    outs = np.stack([br.results[c]["out"] for c in range(NCORES)])
    y = (
        outs.reshape(NCORES, 2, 4, 4, 14, 14, E)
        .transpose(0, 1, 2, 4, 3, 5, 6)
        .reshape(B, H, W, E)
    )
    if _trace:
        return y, br.exec_time_ns
    return y


# revision 19
# speedup vs baseline: 1.2058x; 1.0028x over previous
"""AngularAttention (windowed cosine attention) Trainium2 kernel, 8-core data-parallel.

Per core = 2 images = 32 windows x 196 tokens. See stage comments inline.
"""
import sys

sys.path.insert(0, "/opt/trn_rl_repo")

import numpy as np

import concourse.bacc as bacc
import concourse.mybir as mybir
import concourse.tile as tile
from concourse.bass_utils import run_bass_kernel_spmd

F32 = mybir.dt.float32
F32R = mybir.dt.float32r
BF16 = mybir.dt.bfloat16
AF = mybir.ActivationFunctionType

NCORES = 8
NW = 32
L = 196
T = NW * L
E = 256
SCALE = 10.0

CHUNKS = [(i * 512, 512) for i in range(12)] + [(6144, 128)]
KC = [(0, 128), (128, 68)]


def _build():
    nc = bacc.Bacc(None)
    x = nc.declare_dram_parameter("x", [NW, L, E], F32, isOutput=False)
    w_qkv = nc.declare_dram_parameter("w_qkv", [2, 128, 768], F32R, isOutput=False)
    bqkT = nc.declare_dram_parameter("bqkT", [128, 4], F32, isOutput=False)
    w_proj = nc.declare_dram_parameter("w_proj", [2, 128, 256], F32R, isOutput=False)
    bv_bc = nc.declare_dram_parameter("bv_bc", [128, 256], F32, isOutput=False)
    bp_bc = nc.declare_dram_parameter("bp_bc", [128, 2, 256], F32, isOutput=False)
    ident = nc.declare_dram_parameter("ident", [128, 128], F32, isOutput=False)
    ind16 = nc.declare_dram_parameter("ind16", [128, 4, 128], F32R, isOutput=False)
    bcT = nc.declare_dram_parameter("bcT", [128, 4, 128], F32R, isOutput=False)
    onesb = nc.declare_dram_parameter("onesb", [128, 32], BF16, isOutput=False)
    out = nc.declare_dram_parameter("out", [NW, L, E], F32, isOutput=True)

    with tile.TileContext(nc) as tc:
        from contextlib import ExitStack

        with ExitStack() as root:
            const = root.enter_context(tc.tile_pool(name="const", bufs=1))
            # f32 arena: [0:4] bqkT, [4:132] ident, [132:388] bv, [388:900] bp
            sb_m = const.tile([128, 900], F32)
            nc.sync.dma_start(out=sb_m[:, 0:4], in_=bqkT[:, :])
            nc.sync.dma_start(out=sb_m[:, 4:132], in_=ident[:, :])
            nc.sync.dma_start(out=sb_m[:, 132:388], in_=bv_bc[:, :])
            nc.sync.dma_start(
                out=sb_m[:, 388:900], in_=bp_bc[:, :, :].rearrange("p a b -> p (a b)")
            )
            sb_bqkT = sb_m[:, 0:4]
            sb_id = sb_m[:, 4:132]
            sb_bv = sb_m[:, 132:388]
            sb_bp = sb_m[:, 388:900].rearrange("p (a b) -> p a b", a=2)
            # f32r arena: [0:1536] w_qkv (2,768), [1536:1600] ind16 (4,16), [1600:2112] w_proj (2,256)
            sb_r = const.tile([128, 2560], F32R)
            nc.sync.dma_start(
                out=sb_r[:, 0:1536].rearrange("p (c f) -> p c f", c=2),
                in_=w_qkv[:, :, :].rearrange("c p f -> p c f"),
            )
            nc.sync.dma_start(
                out=sb_r[:, 1536:2048], in_=ind16[:, :, :].rearrange("p a b -> p (a b)")
            )
            nc.sync.dma_start(
                out=sb_r[:, 2048:2560].rearrange("p (c f) -> p c f", c=2),
                in_=w_proj[:, :, :].rearrange("c p f -> p c f"),
            )
            sb_wqkv = sb_r[:, 0:1536].rearrange("p (c f) -> p c f", c=2)
            sb_i16 = sb_r[:, 1536:2048].rearrange("p (a b) -> p a b", a=4)
            sb_wproj = sb_r[:, 2048:2560].rearrange("p (c f) -> p c f", c=2)

            sb_bcT = const.tile([128, 4, 128], F32R)
            nc.sync.dma_start(out=sb_bcT[:], in_=bcT[:, :, :])
            sb_ones = const.tile([128, 32], BF16)
            nc.sync.dma_start(out=sb_ones[:], in_=onesb[:, :])

            big = root.enter_context(tc.tile_pool(name="big", bufs=1))
            qkN = big.tile([128, 4, T], BF16)        # q heads 0-3 | q 4-7 | k 0-3 | k 4-7
            V0 = big.tile([128, NW, 256], BF16)      # k-chunk0 of each window, [tok, (head d)]
            V1 = big.tile([68, NW, 256], BF16)
            Vt = {0: V0, 1: V1}

            # ---------------- Stage A ----------------
            with ExitStack() as sa:
                xin = sa.enter_context(tc.tile_pool(name="xin", bufs=3))
                qkps = sa.enter_context(tc.tile_pool(name="qkps", bufs=2, space="PSUM"))
                ssps = sa.enter_context(tc.tile_pool(name="ssps", bufs=1, space="PSUM"))
                bcps = sa.enter_context(tc.tile_pool(name="bcps", bufs=2, space="PSUM"))
                vps = sa.enter_context(tc.tile_pool(name="vps", bufs=1, space="PSUM"))
                xtp = sa.enter_context(tc.tile_pool(name="xtp", bufs=1))
                rawp = sa.enter_context(tc.tile_pool(name="rawp", bufs=3))
                sqp = sa.enter_context(tc.tile_pool(name="sqp", bufs=2))
                nmp = sa.enter_context(tc.tile_pool(name="nmp", bufs=2))

                xT = xtp.tile([128, 2, T], F32R)

                # A1: load x + PE-transpose, 4 halves (98 tok) per psum batch
                for b0 in range(0, NW * 2, 4):
                    ptr = [
                        qkps.tile([128, 4, 128], F32, tag="qkps", name=f"tr{e}")
                        for e in range(2)
                    ]
                    for s in range(4):
                        hf = b0 + s
                        w, th = hf // 2, hf % 2
                        xt_in = xin.tile([98, 256], F32)
                        nc.sync.dma_start(out=xt_in[:], in_=x[w, 98 * th : 98 * th + 98, :])
                        for e in range(2):
                            nc.tensor.transpose(
                                ptr[e][:, s, 0:98],
                                xt_in[:, 128 * e : 128 * e + 128],
                                sb_id[0:98, 0:98],
                            )
                    for e in range(2):
                        nc.vector.tensor_copy(
                            xT[:, e, 98 * b0 : 98 * (b0 + 4)], ptr[e][:, :, 0:98]
                        )

                # A2: qkT (f32r) + cosine normalization, per token chunk
                for c0, cs in CHUNKS:
                    qk_ps = qkps.tile([128, 2, 512], F32, tag="qkps", name="qkps")
                    ss_ps = ssps.tile([128, 512], F32, tag="ssps", name="ssps")
                    raw = rawp.tile([128, 4, 512], BF16, tag="raw", name="raw")
                    sq = sqp.tile([128, 4, 512], F32R, tag="sq", name="sq")
                    for ft in range(4):
                        slot = ft % 2
                        for e in range(2):
                            nc.tensor.matmul(
                                qk_ps[:, slot, 0:cs],
                                sb_wqkv[:, e, 128 * ft : 128 * ft + 128],
                                xT[:, e, c0 : c0 + cs],
                                start=(e == 0),
                                stop=(e == 1),
                            )
                        nc.scalar.activation(
                            raw[:, ft, 0:cs], qk_ps[:, slot, 0:cs], AF.Identity,
                            bias=sb_bqkT[:, ft : ft + 1], scale=1.0,
                        )
                        sq_eng = nc.gpsimd if ft < 2 else nc.vector
                        sq_eng.tensor_tensor(
                            out=sq[:, ft, 0:cs], in0=raw[:, ft, 0:cs], in1=raw[:, ft, 0:cs],
                            op=mybir.AluOpType.mult,
                        )
                        nc.tensor.matmul(
                            ss_ps[:, 0:cs], sb_i16[:, ft, :], sq[:, ft, 0:cs],
                            start=(ft == 0), stop=(ft == 3),
                        )
                    rss = nmp.tile([128, 512], F32, tag="rss", name="rss")
                    nc.vector.reciprocal_approx_fast(out=rss[:, 0:cs], in_=ss_ps[:, 0:cs])
                    invn = nmp.tile([128, 512], F32R, tag="invn", name="invn")
                    nc.scalar.activation(invn[:, 0:cs], rss[:, 0:cs], AF.Sqrt, scale=1.0)
                    for ft in range(4):
                        bc_ps = bcps.tile([128, 512], F32, tag="bcps", name="bcps")
                        nc.tensor.matmul(
                            bc_ps[:, 0:cs], sb_bcT[:, ft, :], invn[:, 0:cs],
                            start=True, stop=True,
                        )
                        nc.vector.tensor_tensor(
                            out=qkN[:, ft, c0 : c0 + cs], in0=raw[:, ft, 0:cs],
                            in1=bc_ps[:, 0:cs], op=mybir.AluOpType.mult,
                        )

                # A3: V natural (f32r matmuls from xT as stationary)
                for w in range(NW):
                    for ci, (k0, ks) in enumerate(KC):
                        v_ps = vps.tile([128, 256], F32, tag="vps", name="vps")
                        for e in range(2):
                            nc.tensor.matmul(
                                v_ps[0:ks, :],
                                xT[:, e, w * L + k0 : w * L + k0 + ks],
                                sb_wqkv[:, e, 512:768],
                                start=(e == 0), stop=(e == 1),
                            )
                        nc.vector.scalar_tensor_tensor(
                            out=Vt[ci][:, w, :], in0=v_ps[0:ks, :], scalar=1.0,
                            in1=sb_bv[0:ks, :],
                            op0=mybir.AluOpType.mult, op1=mybir.AluOpType.add,
                        )

            # ---------------- Stage B ----------------
            # Quarter passes: B1 = S+exp for 8 windows (PSUM fully dedicated to
            # double-buffered S groups), then B2 = AVT+proj (deep-pipelined).
            etp_ctx = root.enter_context(tc.tile_pool(name="etp", bufs=2))
            invp = root.enter_context(tc.tile_pool(name="invp", bufs=2))
            ysb = root.enter_context(tc.tile_pool(name="ysb", bufs=2))
            etp = etp_ctx
            QW = 8
            Vv = {ci: Vt[ci][:].rearrange("p w (h d) -> p w h d", h=8) for ci in (0, 1)}
            for q in range(NW // QW):
                w0 = q * QW
                with ExitStack() as sq_:
                    expq = sq_.enter_context(tc.tile_pool(name=f"expq{q}", bufs=1))
                    EQ = {
                        ci: expq.tile([ks, QW, 8, L], BF16, tag=f"eq{ci}", name=f"eq{ci}")
                        for ci, (k0, ks) in enumerate(KC)
                    }
                    # ---- B1: S + exp ----
                    with ExitStack() as sb1:
                        sps = sb1.enter_context(tc.tile_pool(name="sps", bufs=2, space="PSUM"))
                        for wl in range(QW):
                            w = w0 + wl
                            for ci, (k0, ks) in enumerate(KC):
                                for g in range(2):
                                    s_ps = sps.tile([128, 4, 512], F32, tag="sps", name="sps")
                                    for hp in range(4):
                                        nc.tensor.matmul(
                                            s_ps[0:ks, hp, 0:L],
                                            qkN[32 * hp : 32 * hp + 32, 2 + g,
                                                w * L + k0 : w * L + k0 + ks],
                                            qkN[32 * hp : 32 * hp + 32, g, w * L : w * L + L],
                                            start=True, stop=True,
                                            tile_position=(32 * hp, 0),
                                        )
                                    nc.scalar.activation(
                                        EQ[ci][:, wl, 4 * g : 4 * g + 4, :],
                                        s_ps[0:ks, :, 0:L],
                                        AF.Exp, scale=SCALE,
                                    )
                    # ---- B2: AVT + proj ----
                    with ExitStack() as sb2:
                        aps = sb2.enter_context(tc.tile_pool(name="aps", bufs=2, space="PSUM"))
                        bps = sb2.enter_context(tc.tile_pool(name="bps", bufs=2, space="PSUM"))
                        yps = sb2.enter_context(tc.tile_pool(name="yps", bufs=2, space="PSUM"))
                        for wp in range(QW // 2):
                            eTs = []
                            for hb in range(2):
                                pA = aps.tile([128, 2, L], F32, tag="pA", name="pA")
                                pB = bps.tile([128, 2, L], F32, tag="pB", name="pB")
                                for wi in range(2):
                                    wl = 2 * wp + wi
                                    w = w0 + wl
                                    for hp in range(4):
                                        h = 4 * hb + hp
                                        for ci, (k0, ks) in enumerate(KC):
                                            nc.tensor.matmul(
                                                pA[32 * hp : 32 * hp + 32, wi, :],
                                                Vv[ci][:, w, h, :],
                                                EQ[ci][:, wl, h, :],
                                                start=(ci == 0), stop=(ci == 1),
                                                tile_position=(0, 32 * hp),
                                            )
                                            nc.tensor.matmul(
                                                pB[32 * hp : 32 * hp + 32, wi, :],
                                                sb_ones[0:ks, :],
                                                EQ[ci][:, wl, h, :],
                                                start=(ci == 0), stop=(ci == 1),
                                                tile_position=(0, 32 * hp),
                                            )
                                inv = invp.tile([128, 2, L], F32, tag="inv", name="inv")
                                nc.vector.reciprocal_approx_fast(out=inv[:], in_=pB[:])
                                eT = etp.tile([128, 2, L], F32R, tag=f"eT{hb}", name=f"eT{hb}")
                                eTs.append(eT)
                                with nc.allow_low_precision(reason="attn out f32r for proj"):
                                    nc.vector.tensor_tensor(
                                        out=eT[:], in0=pA[:], in1=inv[:], op=mybir.AluOpType.mult
                                    )
                            for wi in range(2):
                                w = w0 + 2 * wp + wi
                                y_ps = yps.tile([98, 2, 256], F32, tag="yps", name="yps")
                                for th in range(2):
                                    for hb in range(2):
                                        nc.tensor.matmul(
                                            y_ps[:, th, :],
                                            eTs[hb][:, wi, 98 * th : 98 * th + 98],
                                            sb_wproj[:, hb, :],
                                            start=(hb == 0), stop=(hb == 1),
                                        )
                                y_sb = ysb.tile([98, 2, 256], F32, tag="ysb", name="ysb")
                                nc.vector.scalar_tensor_tensor(
                                    out=y_sb[:], in0=y_ps[:], scalar=1.0, in1=sb_bp[0:98, :, :],
                                    op0=mybir.AluOpType.mult, op1=mybir.AluOpType.add,
                                )
                                nc.sync.dma_start(
                                    out=out[w, :, :].rearrange("(th p) e -> p th e", th=2),
                                    in_=y_sb[:],
                                )

    nc.finalize()
    return nc


_NC = None


def _get_nc():
    global _NC
    if _NC is None:
        _NC = _build()
    return _NC


def _consts():
    import ml_dtypes

    p = np.arange(128)
    ind16 = np.zeros((128, 4, 128), np.float32)
    ind16[:, :, 16:] = 1e-6
    bcT = np.zeros((128, 4, 128), np.float32)
    for ft in range(4):
        ind16[p, ft, 4 * ft + p // 32] = 1.0
        bcT[4 * ft + p // 32, ft, p] = 1.0
    return {
        "ident": np.eye(128, dtype=np.float32),
        "ind16": ind16,
        "bcT": bcT,
        "onesb": np.ones((128, 32), ml_dtypes.bfloat16),
    }


def kernel(x, w_qkv, b_qkv, w_proj, b_proj, _trace=False):
    x = np.ascontiguousarray(np.asarray(x, np.float32))
    w_qkv = np.asarray(w_qkv, np.float32)
    b_qkv = np.asarray(b_qkv, np.float32)
    w_proj = np.asarray(w_proj, np.float32)
    b_proj = np.asarray(b_proj, np.float32)

    B, H, W, _ = x.shape
    xw = (
        x.reshape(NCORES, 2, 4, 14, 4, 14, E)
        .transpose(0, 1, 2, 4, 3, 5, 6)
        .reshape(NCORES, NW, L, E)
    )
    base = {
        "w_qkv": np.ascontiguousarray(w_qkv.reshape(2, 128, 768)),
        "bqkT": np.ascontiguousarray(b_qkv[:512].reshape(4, 128).T),
        "w_proj": np.ascontiguousarray(w_proj.reshape(2, 128, 256)),
        "bv_bc": np.broadcast_to(b_qkv[512:768], (128, 256)).copy(),
        "bp_bc": np.broadcast_to(b_proj, (128, 2, 256)).copy(),
        **_consts(),
    }
    in_maps = [dict(base, x=np.ascontiguousarray(xw[c])) for c in range(NCORES)]

    nc = _get_nc()
    br = run_bass_kernel_spmd(nc, in_maps, list(range(NCORES)), trace=_trace)
    outs = np.stack([br.results[c]["out"] for c in range(NCORES)])
    y = (
        outs.reshape(NCORES, 2, 4, 4, 14, 14, E)
        .transpose(0, 1, 2, 4, 3, 5, 6)
        .reshape(B, H, W, E)
    )
    if _trace:
        return y, br.exec_time_ns
    return y


# revision 20
# speedup vs baseline: 1.2155x; 1.0080x over previous
"""AngularAttention (windowed cosine attention) Trainium2 kernel, 8-core data-parallel.

Per core = 2 images = 32 windows x 196 tokens. See stage comments inline.
"""
import sys

sys.path.insert(0, "/opt/trn_rl_repo")

import numpy as np

import concourse.bacc as bacc
import concourse.mybir as mybir
import concourse.tile as tile
from concourse.bass_utils import run_bass_kernel_spmd

F32 = mybir.dt.float32
F32R = mybir.dt.float32r
BF16 = mybir.dt.bfloat16
AF = mybir.ActivationFunctionType

NCORES = 8
NW = 32
L = 196
T = NW * L
E = 256
SCALE = 10.0

CHUNKS = [(i * 512, 512) for i in range(12)] + [(6144, 128)]
KC = [(0, 128), (128, 68)]


def _build():
    nc = bacc.Bacc(None)
    x = nc.declare_dram_parameter("x", [NW, L, E], F32, isOutput=False)
    w_qkv = nc.declare_dram_parameter("w_qkv", [2, 128, 768], F32R, isOutput=False)
    bqkT = nc.declare_dram_parameter("bqkT", [128, 4], F32, isOutput=False)
    w_proj = nc.declare_dram_parameter("w_proj", [2, 128, 256], F32R, isOutput=False)
    bv_bc = nc.declare_dram_parameter("bv_bc", [128, 256], F32, isOutput=False)
    bp_bc = nc.declare_dram_parameter("bp_bc", [128, 2, 256], F32, isOutput=False)
    ident = nc.declare_dram_parameter("ident", [128, 128], F32, isOutput=False)
    ind16 = nc.declare_dram_parameter("ind16", [128, 4, 128], F32R, isOutput=False)
    bcT = nc.declare_dram_parameter("bcT", [128, 4, 128], F32R, isOutput=False)
    onesb = nc.declare_dram_parameter("onesb", [128, 32], BF16, isOutput=False)
    out = nc.declare_dram_parameter("out", [NW, L, E], F32, isOutput=True)

    with tile.TileContext(nc) as tc:
        from contextlib import ExitStack

        with ExitStack() as root:
            const = root.enter_context(tc.tile_pool(name="const", bufs=1))
            # f32 arena: [0:4] bqkT, [4:132] ident, [132:388] bv, [388:900] bp
            sb_m = const.tile([128, 900], F32)
            nc.sync.dma_start(out=sb_m[:, 0:4], in_=bqkT[:, :])
            nc.sync.dma_start(out=sb_m[:, 4:132], in_=ident[:, :])
            nc.sync.dma_start(out=sb_m[:, 132:388], in_=bv_bc[:, :])
            nc.sync.dma_start(
                out=sb_m[:, 388:900], in_=bp_bc[:, :, :].rearrange("p a b -> p (a b)")
            )
            sb_bqkT = sb_m[:, 0:4]
            sb_id = sb_m[:, 4:132]
            sb_bv = sb_m[:, 132:388]
            sb_bp = sb_m[:, 388:900].rearrange("p (a b) -> p a b", a=2)
            # f32r arena: [0:1536] w_qkv (2,768), [1536:1600] ind16 (4,16), [1600:2112] w_proj (2,256)
            sb_r = const.tile([128, 2560], F32R)
            nc.sync.dma_start(
                out=sb_r[:, 0:1536].rearrange("p (c f) -> p c f", c=2),
                in_=w_qkv[:, :, :].rearrange("c p f -> p c f"),
            )
            nc.sync.dma_start(
                out=sb_r[:, 1536:2048], in_=ind16[:, :, :].rearrange("p a b -> p (a b)")
            )
            nc.sync.dma_start(
                out=sb_r[:, 2048:2560].rearrange("p (c f) -> p c f", c=2),
                in_=w_proj[:, :, :].rearrange("c p f -> p c f"),
            )
            sb_wqkv = sb_r[:, 0:1536].rearrange("p (c f) -> p c f", c=2)
            sb_i16 = sb_r[:, 1536:2048].rearrange("p (a b) -> p a b", a=4)
            sb_wproj = sb_r[:, 2048:2560].rearrange("p (c f) -> p c f", c=2)

            sb_bcT = const.tile([128, 4, 128], F32R)
            nc.sync.dma_start(out=sb_bcT[:], in_=bcT[:, :, :])
            sb_ones = const.tile([128, 32], BF16)
            nc.sync.dma_start(out=sb_ones[:], in_=onesb[:, :])

            big = root.enter_context(tc.tile_pool(name="big", bufs=1))
            qkN = big.tile([128, 4, T], BF16)        # q heads 0-3 | q 4-7 | k 0-3 | k 4-7
            V0 = big.tile([128, NW, 256], BF16)      # k-chunk0 of each window, [tok, (head d)]
            V1 = big.tile([68, NW, 256], BF16)
            Vt = {0: V0, 1: V1}

            # ---------------- Stage A ----------------
            with ExitStack() as sa:
                xin = sa.enter_context(tc.tile_pool(name="xin", bufs=3))
                qkps = sa.enter_context(tc.tile_pool(name="qkps", bufs=2, space="PSUM"))
                ssps = sa.enter_context(tc.tile_pool(name="ssps", bufs=1, space="PSUM"))
                bcps = sa.enter_context(tc.tile_pool(name="bcps", bufs=2, space="PSUM"))
                vps = sa.enter_context(tc.tile_pool(name="vps", bufs=1, space="PSUM"))
                xtp = sa.enter_context(tc.tile_pool(name="xtp", bufs=1))
                rawp = sa.enter_context(tc.tile_pool(name="rawp", bufs=3))
                sqp = sa.enter_context(tc.tile_pool(name="sqp", bufs=2))
                nmp = sa.enter_context(tc.tile_pool(name="nmp", bufs=2))

                xT = xtp.tile([128, 2, T], F32R)

                # A1: load x + PE-transpose, 4 halves (98 tok) per psum batch
                for b0 in range(0, NW * 2, 4):
                    ptr = [
                        qkps.tile([128, 4, 128], F32, tag="qkps", name=f"tr{e}")
                        for e in range(2)
                    ]
                    for s in range(4):
                        hf = b0 + s
                        w, th = hf // 2, hf % 2
                        xt_in = xin.tile([98, 256], F32)
                        nc.sync.dma_start(out=xt_in[:], in_=x[w, 98 * th : 98 * th + 98, :])
                        for e in range(2):
                            nc.tensor.transpose(
                                ptr[e][:, s, 0:98],
                                xt_in[:, 128 * e : 128 * e + 128],
                                sb_id[0:98, 0:98],
                            )
                    for e in range(2):
                        nc.vector.tensor_copy(
                            xT[:, e, 98 * b0 : 98 * (b0 + 4)], ptr[e][:, :, 0:98]
                        )

                # A2: qkT (f32r) + cosine normalization, per token chunk
                for c0, cs in list(reversed(CHUNKS)):
                    qk_ps = qkps.tile([128, 2, 512], F32, tag="qkps", name="qkps")
                    ss_ps = ssps.tile([128, 512], F32, tag="ssps", name="ssps")
                    raw = rawp.tile([128, 4, 512], BF16, tag="raw", name="raw")
                    sq = sqp.tile([128, 4, 512], F32R, tag="sq", name="sq")
                    for ft in range(4):
                        slot = ft % 2
                        for e in range(2):
                            nc.tensor.matmul(
                                qk_ps[:, slot, 0:cs],
                                sb_wqkv[:, e, 128 * ft : 128 * ft + 128],
                                xT[:, e, c0 : c0 + cs],
                                start=(e == 0),
                                stop=(e == 1),
                            )
                        nc.scalar.activation(
                            raw[:, ft, 0:cs], qk_ps[:, slot, 0:cs], AF.Identity,
                            bias=sb_bqkT[:, ft : ft + 1], scale=1.0,
                        )
                        sq_eng = nc.gpsimd if ft < 3 else nc.vector
                        sq_eng.tensor_tensor(
                            out=sq[:, ft, 0:cs], in0=raw[:, ft, 0:cs], in1=raw[:, ft, 0:cs],
                            op=mybir.AluOpType.mult,
                        )
                        nc.tensor.matmul(
                            ss_ps[:, 0:cs], sb_i16[:, ft, :], sq[:, ft, 0:cs],
                            start=(ft == 0), stop=(ft == 3),
                        )
                    rss = nmp.tile([128, 512], F32, tag="rss", name="rss")
                    nc.vector.reciprocal_approx_fast(out=rss[:, 0:cs], in_=ss_ps[:, 0:cs])
                    invn = nmp.tile([128, 512], F32R, tag="invn", name="invn")
                    nc.scalar.activation(invn[:, 0:cs], rss[:, 0:cs], AF.Sqrt, scale=1.0)
                    for ft in range(4):
                        bc_ps = bcps.tile([128, 512], F32, tag="bcps", name="bcps")
                        nc.tensor.matmul(
                            bc_ps[:, 0:cs], sb_bcT[:, ft, :], invn[:, 0:cs],
                            start=True, stop=True,
                        )
                        nc.vector.tensor_tensor(
                            out=qkN[:, ft, c0 : c0 + cs], in0=raw[:, ft, 0:cs],
                            in1=bc_ps[:, 0:cs], op=mybir.AluOpType.mult,
                        )

                # A3: V natural (f32r matmuls from xT as stationary)
                for w in range(NW):
                    for ci, (k0, ks) in enumerate(KC):
                        v_ps = vps.tile([128, 256], F32, tag="vps", name="vps")
                        for e in range(2):
                            nc.tensor.matmul(
                                v_ps[0:ks, :],
                                xT[:, e, w * L + k0 : w * L + k0 + ks],
                                sb_wqkv[:, e, 512:768],
                                start=(e == 0), stop=(e == 1),
                            )
                        nc.vector.scalar_tensor_tensor(
                            out=Vt[ci][:, w, :], in0=v_ps[0:ks, :], scalar=1.0,
                            in1=sb_bv[0:ks, :],
                            op0=mybir.AluOpType.mult, op1=mybir.AluOpType.add,
                        )

            # ---------------- Stage B ----------------
            # Quarter passes: B1 = S+exp for 8 windows (PSUM fully dedicated to
            # double-buffered S groups), then B2 = AVT+proj (deep-pipelined).
            etp_ctx = root.enter_context(tc.tile_pool(name="etp", bufs=2))
            invp = root.enter_context(tc.tile_pool(name="invp", bufs=2))
            ysb = root.enter_context(tc.tile_pool(name="ysb", bufs=2))
            etp = etp_ctx
            QW = 8
            Vv = {ci: Vt[ci][:].rearrange("p w (h d) -> p w h d", h=8) for ci in (0, 1)}
            for q in range(NW // QW):
                w0 = q * QW
                with ExitStack() as sq_:
                    expq = sq_.enter_context(tc.tile_pool(name=f"expq{q}", bufs=1))
                    EQ = {
                        ci: expq.tile([ks, QW, 8, L], BF16, tag=f"eq{ci}", name=f"eq{ci}")
                        for ci, (k0, ks) in enumerate(KC)
                    }
                    # ---- B1: S + exp ----
                    with ExitStack() as sb1:
                        sps = sb1.enter_context(tc.tile_pool(name="sps", bufs=2, space="PSUM"))
                        for wl in range(QW):
                            w = w0 + wl
                            for ci, (k0, ks) in enumerate(KC):
                                for g in range(2):
                                    s_ps = sps.tile([128, 4, 512], F32, tag="sps", name="sps")
                                    for hp in range(4):
                                        nc.tensor.matmul(
                                            s_ps[0:ks, hp, 0:L],
                                            qkN[32 * hp : 32 * hp + 32, 2 + g,
                                                w * L + k0 : w * L + k0 + ks],
                                            qkN[32 * hp : 32 * hp + 32, g, w * L : w * L + L],
                                            start=True, stop=True,
                                            tile_position=(32 * hp, 0),
                                        )
                                    nc.scalar.activation(
                                        EQ[ci][:, wl, 4 * g : 4 * g + 4, :],
                                        s_ps[0:ks, :, 0:L],
                                        AF.Exp, scale=SCALE,
                                    )
                    # ---- B2: AVT + proj ----
                    with ExitStack() as sb2:
                        aps = sb2.enter_context(tc.tile_pool(name="aps", bufs=2, space="PSUM"))
                        bps = sb2.enter_context(tc.tile_pool(name="bps", bufs=2, space="PSUM"))
                        yps = sb2.enter_context(tc.tile_pool(name="yps", bufs=2, space="PSUM"))
                        for wp in range(QW // 2):
                            eTs = []
                            for hb in range(2):
                                pA = aps.tile([128, 2, L], F32, tag="pA", name="pA")
                                pB = bps.tile([128, 2, L], F32, tag="pB", name="pB")
                                for wi in range(2):
                                    wl = 2 * wp + wi
                                    w = w0 + wl
                                    for hp in range(4):
                                        h = 4 * hb + hp
                                        for ci, (k0, ks) in enumerate(KC):
                                            nc.tensor.matmul(
                                                pA[32 * hp : 32 * hp + 32, wi, :],
                                                Vv[ci][:, w, h, :],
                                                EQ[ci][:, wl, h, :],
                                                start=(ci == 0), stop=(ci == 1),
                                                tile_position=(0, 32 * hp),
                                            )
                                            nc.tensor.matmul(
                                                pB[32 * hp : 32 * hp + 32, wi, :],
                                                sb_ones[0:ks, :],
                                                EQ[ci][:, wl, h, :],
                                                start=(ci == 0), stop=(ci == 1),
                                                tile_position=(0, 32 * hp),
                                            )
                                inv = invp.tile([128, 2, L], F32, tag="inv", name="inv")
                                nc.vector.reciprocal_approx_fast(out=inv[:], in_=pB[:])
                                eT = etp.tile([128, 2, L], F32R, tag=f"eT{hb}", name=f"eT{hb}")
                                eTs.append(eT)
                                with nc.allow_low_precision(reason="attn out f32r for proj"):
                                    nc.vector.tensor_tensor(
                                        out=eT[:], in0=pA[:], in1=inv[:], op=mybir.AluOpType.mult
                                    )
                            for wi in range(2):
                                w = w0 + 2 * wp + wi
                                y_ps = yps.tile([98, 2, 256], F32, tag="yps", name="yps")
                                for th in range(2):
                                    for hb in range(2):
                                        nc.tensor.matmul(
                                            y_ps[:, th, :],
                                            eTs[hb][:, wi, 98 * th : 98 * th + 98],
                                            sb_wproj[:, hb, :],
                                            start=(hb == 0), stop=(hb == 1),
                                        )
                                y_sb = ysb.tile([98, 2, 256], F32, tag="ysb", name="ysb")
                                nc.vector.scalar_tensor_tensor(
                                    out=y_sb[:], in0=y_ps[:], scalar=1.0, in1=sb_bp[0:98, :, :],
                                    op0=mybir.AluOpType.mult, op1=mybir.AluOpType.add,
                                )
                                nc.sync.dma_start(
                                    out=out[w, :, :].rearrange("(th p) e -> p th e", th=2),
                                    in_=y_sb[:],
                                )

    nc.finalize()
    return nc


_NC = None


def _get_nc():
    global _NC
    if _NC is None:
        _NC = _build()
    return _NC


def _consts():
    import ml_dtypes

    p = np.arange(128)
    ind16 = np.zeros((128, 4, 128), np.float32)
    ind16[:, :, 16:] = 1e-6
    bcT = np.zeros((128, 4, 128), np.float32)
    for ft in range(4):
        ind16[p, ft, 4 * ft + p // 32] = 1.0
        bcT[4 * ft + p // 32, ft, p] = 1.0
    return {
        "ident": np.eye(128, dtype=np.float32),
        "ind16": ind16,
        "bcT": bcT,
        "onesb": np.ones((128, 32), ml_dtypes.bfloat16),
    }


def kernel(x, w_qkv, b_qkv, w_proj, b_proj, _trace=False):
    x = np.ascontiguousarray(np.asarray(x, np.float32))
    w_qkv = np.asarray(w_qkv, np.float32)
    b_qkv = np.asarray(b_qkv, np.float32)
    w_proj = np.asarray(w_proj, np.float32)
    b_proj = np.asarray(b_proj, np.float32)

    B, H, W, _ = x.shape
    xw = (
        x.reshape(NCORES, 2, 4, 14, 4, 14, E)
        .transpose(0, 1, 2, 4, 3, 5, 6)
        .reshape(NCORES, NW, L, E)
    )
    base = {
        "w_qkv": np.ascontiguousarray(w_qkv.reshape(2, 128, 768)),
        "bqkT": np.ascontiguousarray(b_qkv[:512].reshape(4, 128).T),
        "w_proj": np.ascontiguousarray(w_proj.reshape(2, 128, 256)),
        "bv_bc": np.broadcast_to(b_qkv[512:768], (128, 256)).copy(),
        "bp_bc": np.broadcast_to(b_proj, (128, 2, 256)).copy(),
        **_consts(),
    }
    in_maps = [dict(base, x=np.ascontiguousarray(xw[c])) for c in range(NCORES)]

    nc = _get_nc()
    br = run_bass_kernel_spmd(nc, in_maps, list(range(NCORES)), trace=_trace)
    outs = np.stack([br.results[c]["out"] for c in range(NCORES)])
    y = (
        outs.reshape(NCORES, 2, 4, 4, 14, 14, E)
        .transpose(0, 1, 2, 4, 3, 5, 6)
        .reshape(B, H, W, E)
    )
    if _trace:
        return y, br.exec_time_ns
    return y
